# revision 84
# baseline (speedup 1.0000x reference)
"""
Trainium2 Bass kernel for nn_MultiHeadHierarchicalAttention.

Sharding: 8 cores = (batch b in 0..3) x (block-half in 0..1).
Each core handles one batch and 16 of the 32 blocks for the token-level
attention; the (small) sentence-level attention + fc1 branch is computed
redundantly on both cores of a batch, scaled by 0.5, and the host sums the
two per-batch partial outputs (the final fc is linear, so partial ctx_w
contributions simply add).

Pipeline: (head-pair, block-pair) main loop in TWO SWEEPS (each head-pair
covers block-pairs 0-3 in sweep one, 4-7 in sweep two) so the sg2/sg3
kw/vw projections spread across the loop middle instead of piling into
head-pair 0; skew-2 scores->exp->s3; Scalar engine reserved for exp (the
~1.07us/tile exp stream is the kernel's wall); ctx accumulation on GpSimd;
deferred sentence branch overlapping the startup DMA trickle; final fc
emitted after the loop so it ranks below the tail chain.

Key tricks:
  - score matmuls are K=64 row-tiled PAIRS (tile_position 0/64) running
    concurrently in the PE array (~2x vs serial K=128 with padded qw).
  - fp8 weights (W_kw/W_vw/W_qw/W_qs/W_ks) are x16-scaled on the host so
    std-0.02 entries stay in e4m3's normal range; the 1/256 is folded into
    the exp scale and the vw ones-column is 16.0 so the softmax factor
    self-corrects. Mixed fp8-lhsT x bf16-rhs matmuls are used directly.
  - startup DMAs ride the sync+scalar hardware rings (FIFO = priority,
    ~215 GB/s pooled) with few triggers (~0.7us each); small bias loads go
    before bulky weights on gpsimd's ~85 GB/s software queue.

Device layouts (per core, partition dim first):
  qT   [D, LQ]      kwT/vwT [D, 4096]   ksT/vsT [D, 32] (block-rolled)
  projections keep features on partitions (kw) or tokens on partitions (vw)
  token scores are computed as [t, q] tiles (K=dk=64), exp'd on ACT into
  bf16, and consumed per block by S3 matmuls producing [q, dv+1] partials
  (the +1 "ones" column of vw gives the softmax denominator for free).
  The sentence-attention factor attn_s/denom scales the per-block ctx on
  DVE; b_vw is folded into vw_sb so no separate bias pass is needed.
  ctx_w is PE-transposed at the end and fused into the final fc, which is
  emitted as outT [D, LQ] (host transposes and sums core pairs).
"""

import sys

sys.path.insert(0, "/opt/trn_rl_repo")

import numpy as np
import ml_dtypes
import concourse.bass as bass
import concourse.tile as tile
from concourse import mybir
from concourse.bass_utils import run_bass_kernel_spmd
from concourse.vector_clock import ScopedClock
from concourse.masks import make_identity

# ---------------------------------------------------------------- constants
B, LQ, NB, NT = 4, 512, 32, 256
D, H, DK, DV = 512, 8, 64, 64
NBH = NB // 2  # blocks per core
NTOK = NBH * NT  # tokens per core = 4096
NTC = NTOK // 128  # 32 token chunks of 128
SCALE = 0.125
# fp8 weights are scaled x16 on the host so W std ~0.02 lands in e4m3's
# normal range (raw 0.02-std weights round through subnormals at huge
# relative error). The projections then produce 16x-scaled kw/qw/qs/ks;
# evictions and exp scales fold the 1/16 back in.
WSC = 16.0
TOK_EXP_SCALE = SCALE / (WSC * WSC)  # qw and kw both carry x16
# Schraudolph fast-exp (bf16-as-int16): exp(s*x) ~= bitcast(i16(A*x+B))
EXP_A = 128 * 1.4426950408889634 * TOK_EXP_SCALE
EXP_B = 16250.5
FP = mybir.dt.float32
FR = mybir.dt.float32r
BF = mybir.dt.bfloat16
F8 = mybir.dt.float8e4
N_CORES = 8

AX = mybir.AxisListType.X
ALU = mybir.AluOpType
ACTF = mybir.ActivationFunctionType


# --------------------------------------------------------- drain workaround
def _patched_drain_and_barrier(self, tick_clock, wait_clock):
    # walrus in this container rejects >1 sem wait on a single TPB_CTRL
    # instruction ("Too many sync wait commands"); split the kernel-tail
    # drain waits across one-wait NOPs.
    nop_inst = self.nc.sync.nop(nofuse=True)
    wait_clock.add_sem_waits(nop_inst.ins, ScopedClock({None: tick_clock.global_clock}))
    waits = list(nop_inst.ins.sync_info.on_wait or [])
    if len(waits) > 1:
        nop_inst.ins.sync_info.on_wait = waits[:1]
        rest = waits[1:]
        while rest:
            extra = self.nc.sync.nop(nofuse=True)
            if extra.ins.sync_info is None:
                extra.ins.sync_info = mybir.SyncInfo(on_wait=[], on_update=[])
            extra.ins.sync_info.on_wait = rest[:1]
            rest = rest[1:]
    self.nc.sync.drain()
    self.nc.all_engine_barrier()
    assert self.sems is not None
    popped = self.nc._tile_sem_poison_stack.pop()
    assert popped is self._sem_poison
    self.nc.clear_and_free_semaphores(list(self.sems.allocated().values()))
    self.nc.all_engine_barrier()


_ORIG_DRAIN_AND_BARRIER = tile.TileContext._drain_and_barrier
tile.TileContext._drain_and_barrier = _patched_drain_and_barrier


def _r(ap):
    """View an f32 AP as float32r so matmuls run at 1 cycle/row."""
    return ap.bitcast(mybir.dt.float32r)


_NO_SPLIT_OPCODES = {
    "CollectiveCompute",
    "EventSemaphore",
}
_split_counter = [0]


def _split_multi_waits(nc):
    """This container's walrus accepts at most ONE sem wait per TPB
    instruction; hoist extra waits onto same-engine NOPs placed before."""
    n_split = 0
    for fn in nc.m.functions:
        for bb in fn.blocks:
            changed = False
            out = []
            for inst in bb.instructions:
                si = inst.sync_info
                if (
                    si is not None
                    and si.on_wait
                    and len(list(si.on_wait)) > 1
                    and inst.opcode not in _NO_SPLIT_OPCODES
                ):
                    waits = list(si.on_wait)
                    for w in waits[:-1]:
                        _split_counter[0] += 1
                        nop = mybir.InstNoOp(name=f"I-wsplit-{_split_counter[0]}")
                        nop.engine = inst.engine
                        nop.sync_info = mybir.SyncInfo(on_wait=[w], on_update=[])
                        out.append(nop)
                        n_split += 1
                    si.on_wait = waits[-1:]
                    changed = True
                out.append(inst)
            if changed:
                bb.instructions = out
    return n_split


def _flat2(ap):
    """[p, a, b] -> [p, a*b]"""
    return ap.rearrange("p a b -> p (a b)")


# ------------------------------------------------------------ program build
def build_program(for_sim=False):
    # the walrus-only wait-splitting workarounds confuse CoreSim's race
    # detector; skip them when building for simulation.
    tile.TileContext._drain_and_barrier = (
        _ORIG_DRAIN_AND_BARRIER if for_sim else _patched_drain_and_barrier
    )
    nc = bass.Bass("TRN2", target_bir_lowering=False, debug=False, num_devices=N_CORES)

    dt_in = {}
    for name, shape in [
        ("kwT", [4, 128, NTOK]),
        ("vwT", [4, 128, NTOK]),
        ("Wkw", [128, 4, H * DK]),
        ("Wvw", [128, 4, H * DV]),
        ("Wqs", [128, 4, H * DK]),
        ("Wks", [128, 4, H * DK]),
        ("Wqw", [128, 4, H * DK]),
    ]:
        dt_in[name] = nc.dram_tensor(name, shape, F8, kind="ExternalInput").ap()
    for name, shape in [
        ("qT", [128, 4, LQ]),
        ("ksT", [128, 4, NB]),
        ("vsT", [128, 4, NB]),
        ("Wvs", [128, 4, H * DV]),
        ("Wfc", [128, 8, D]),
        ("Wfc1", [128, 4, D]),
    ]:
        dt_in[name] = nc.dram_tensor(name, shape, BF, kind="ExternalInput").ap()
    for name, shape in [
        ("bqsT", [128, 4]),
        ("bksT", [128, 4]),
        ("bqwT", [128, 4]),
        ("bkwT", [128, 4]),
        ("bvsT", [128, 4]),
        ("bfc1T", [128, 4]),
        ("bfcT", [128, 4]),
        ("bvw", [H * DV]),
    ]:
        dt_in[name] = nc.dram_tensor(name, shape, FP, kind="ExternalInput").ap()
    outT_d = nc.dram_tensor("outT", [D, LQ], BF, kind="ExternalOutput").ap()

    with tile.TileContext(nc) as tc:
        # ------------------------------------------------ persistent pools
        ppool_cm = tc.tile_pool(name="persist", bufs=1)
        ppool = ppool_cm.__enter__()
        scpool_cm = tc.tile_pool(name="scps", bufs=2, space="PSUM")
        scpool = scpool_cm.__enter__()
        s3pool_cm = tc.tile_pool(name="s3ps", bufs=2, space="PSUM")
        s3pool = s3pool_cm.__enter__()
        ewpool_cm = tc.tile_pool(name="ewp", bufs=18)
        ewpool = ewpool_cm.__enter__()
        smpool_cm = tc.tile_pool(name="small", bufs=8)
        smpool = smpool_cm.__enter__()

        ident = ppool.tile([128, 128], FP, tag="ident")

        # persistent sbuf tensors
        qw_pad = ppool.tile([128, H, LQ], BF, tag="qw_pad")
        ks_sb = ppool.tile([128, 4, NB], BF, tag="ks_sb")
        attn_sb = ppool.tile([128, 4, H, NB], FP, tag="attn_sb")
        fc1T_sb = ppool.tile([128, 4, LQ], BF, tag="fc1T")
        kw_sb = ppool.tile([128, 4, NTOK], BF, tag="kw_sb")
        vw_sb = ppool.tile([128, NTC, H, DV + 1], BF, tag="vw_sb")
        ctx_acc = ppool.tile([128, 4, H * DV], FP, tag="ctx_acc")
        vs_sb = ppool.tile([NB, H * DV], BF, tag="vs_sb")
        ctx_sT = ppool.tile([128, 4, LQ], BF, tag="ctx_sT")

        # staging pool + kw/vw weight tiles created early so the big DMAs
        # stream during the small branch.
        stpool_cm = tc.tile_pool(name="stage", bufs=3)
        stpool = stpool_cm.__enter__()
        Wkw_sb = ppool.tile([128, 4, H * DK], F8, tag="Wkw")
        Wvw_sb = ppool.tile([128, 4, H * DV], F8, tag="Wvw")
        ctx_wT = ppool.tile([128, 4, LQ], BF, tag="ctx_wT")
        stg_store = {}

        def dma_halves(tile4, in_ap, eng0, eng1):
            # split a [128, 4, n] load across two DMA queues: one queue tops
            # out around ~70 GB/s, which serializes the startup badly.
            eng0.dma_start(out=tile4[:, 0:2, :], in_=in_ap[:, 0:2, :])
            eng1.dma_start(out=tile4[:, 2:4, :], in_=in_ap[:, 2:4, :])

        def stage_dma(sg, engs=None):
            kstg = stpool.tile([128, 4, 1024], F8, tag="kstg", name="kstg")
            vstg = stpool.tile([128, 4, 1024], F8, tag="vstg", name="vstg")
            if engs is None:
                nc.sync.dma_start(
                    out=kstg.rearrange("p a b -> p (a b)"), in_=dt_in["kwT"][sg]
                )
                nc.sync.dma_start(
                    out=vstg.rearrange("p a b -> p (a b)"), in_=dt_in["vwT"][sg]
                )
            else:
                ka = dt_in["kwT"][sg].rearrange("p (a b) -> p a b", b=1024)
                va = dt_in["vwT"][sg].rearrange("p (a b) -> p a b", b=1024)
                dma_halves(kstg, ka, engs[0], engs[1])
                dma_halves(vstg, va, engs[2], engs[3])
            stg_store[sg] = (kstg, vstg)

        DR = mybir.MatmulPerfMode.DoubleRow

        def kw_compute(sg):
            stg = stg_store[sg][0]
            for mo in range(4):
                ps = s3pool.tile([128, 2, 512], FP, tag="s3", name="kwps")
                for j in range(2):
                    for s in range(2):
                        nc.tensor.matmul(
                            ps[:, j, :],
                            Wkw_sb[:, 2 * s : 2 * s + 2, mo * 128 : (mo + 1) * 128],
                            stg[:, 2 * s : 2 * s + 2, j * 512 : (j + 1) * 512],
                            start=(s == 0),
                            stop=(s == 1),
                            perf_mode=DR,
                        )
                # NOTE: b_kw adds a per-(h,q) constant to every score in every
                # block; it cancels in the within-block softmax and in the
                # attn_s/denom ratio, so it is dropped entirely.
                if sg == 0:
                    # startup: ACT is idle until the first exp (~25us), so
                    # the sg0 evictions run there, off the DVE critical
                    # path that gates the first score matmuls
                    nc.scalar.activation(
                        kw_sb[:, mo, sg * 1024 : (sg + 1) * 1024],
                        _flat2(ps),
                        ACTF.Copy,
                    )
                else:
                    nc.vector.tensor_copy(
                        kw_sb[:, mo, sg * 1024 : (sg + 1) * 1024], _flat2(ps)
                    )

        def vw_compute(sg):
            stg = stg_store[sg][1]
            for tp in range(4):
                ps = s3pool.tile([128, 2, 512], FP, tag="s3", name="vwps")
                for j in range(2):
                    tcl = tp * 2 + j
                    for s in range(2):
                        nc.tensor.matmul(
                            ps[:, j, :],
                            stg[:, 2 * s : 2 * s + 2, tcl * 128 : (tcl + 1) * 128],
                            Wvw_sb[:, 2 * s : 2 * s + 2, :],
                            start=(s == 0),
                            stop=(s == 1),
                            perf_mode=DR,
                        )
                tc0 = sg * 8 + tp * 2
                # fold b_vw into vw: the per-block softmax-weighted sum of
                # (vw + b) equals ctx + denom*b, and the attn_s/denom factor
                # turns that into exactly the b_vw bias term summed over
                # blocks — no separate head-tail bias pass needed.
                bap = bvw_bc[:].rearrange("p (h x) -> p h x", x=DV)
                bvw_view = bass.AP(
                    bap.tensor,
                    bap.offset,
                    [list(bap.ap[0])] + [[0, 2]] + [list(d) for d in bap.ap[1:]],
                )
                nc.vector.tensor_tensor(
                    vw_sb[:, tc0 : tc0 + 2, :, 0:DV],
                    ps.rearrange("p a (h x) -> p a h x", x=DV),
                    bvw_view,
                    ALU.add,
                )

        # ------------------------------------------- phase 1: token-path start
        # Critical path: qT+Wqw -> qw proj -> kw sg0 -> first scores -> exp.
        # The sentence branch is deferred into the first two main-loop
        # iterations so ACT starts exp'ing as early as possible.
        apool_cm = tc.tile_pool(name="phaseA", bufs=1)
        apool = apool_cm.__enter__()

        qT_sb = apool.tile([128, 4, LQ], BF, tag="qT_sb")
        ksT_sb = apool.tile([128, 4, NB], BF, tag="ksT_sb")
        vsT_sb = apool.tile([128, 4, NB], BF, tag="vsT_sb")
        qs_sb = apool.tile([128, 4, LQ], BF, tag="qs_sb")

        def load_W(wn, wdt=BF, eng=None, engs=None):
            t = apool.tile([128, 4, D], wdt, tag=f"W_{wn}", name=wn)
            if engs is not None:
                dma_halves(t, dt_in[wn], engs[0], engs[1])
            else:
                (eng or nc.gpsimd).dma_start(out=t[:], in_=dt_in[wn][:])
            return t

        # Startup DMAs. Each dma_start trigger costs ~0.7us on the issuing
        # engine, and transfers stripe across the 16 DMA engines regardless,
        # so: few triggers, hardware ring (sync) drains ~215 GB/s FIFO in
        # emission order = priority, gpsimd's software queue (~85 GB/s) runs
        # the fp8 weights + small loads concurrently. Nothing on scalar --
        # the exp wall lives there.
        # the scalar engine is idle until the first exp (~20us), so its DMA
        # ring carries half of the startup-critical loads for free
        dma_halves(qT_sb, dt_in["qT"], nc.sync, nc.scalar)
        Wqw_t = load_W("Wqw", F8)
        # tiny loads (biases feed the qw/qs evictions!) must not queue behind
        # the bulky fp8 weights on gpsimd's slow software queue
        btiles = {}
        for bn in ["bqsT", "bqwT", "bvsT", "bfc1T", "bfcT"]:
            t = ppool.tile([128, 4], FP, tag=bn)
            nc.gpsimd.dma_start(out=t[:], in_=dt_in[bn][:])
            btiles[bn] = t
        bvw_bc = ppool.tile([128, H * DV], FP, tag="bvw_bc")
        src = dt_in["bvw"]
        bcast_ap = bass.AP(src.tensor, src.offset, [[0, 128]] + [list(x) for x in src.ap])
        nc.gpsimd.dma_start(out=bvw_bc[:], in_=bcast_ap)
        nc.gpsimd.dma_start(out=ksT_sb[:], in_=dt_in["ksT"][:])
        nc.gpsimd.dma_start(out=vsT_sb[:], in_=dt_in["vsT"][:])
        nc.sync.dma_start(out=Wkw_sb[:], in_=dt_in["Wkw"][:])
        stage_dma(0, engs=(nc.sync, nc.scalar, nc.sync, nc.scalar))
        nc.sync.dma_start(out=Wvw_sb[:], in_=dt_in["Wvw"][:])
        Wqs_t = load_W("Wqs", F8, eng=nc.sync)
        Wks_t = load_W("Wks", F8, eng=nc.sync)
        make_identity(nc, ident)
        nc.vector.memset(vw_sb[:, :, :, DV : DV + 1], WSC)
        # PE warm-up: ~4us of dummy matmuls while the startup DMAs stream,
        # so the HAM clock-gate reaches 2.4 GHz before the real projections
        # (otherwise qw/kw0 run at the cold 1.2 GHz rate)
        wu_sb = apool.tile([128, 128], BF, tag="warmup")
        nc.vector.memset(wu_sb[:], 0.0)
        wu_ps = scpool.tile([128, 512], FP, tag="sc", name="wups")
        for i in range(32):
            nc.tensor.matmul(
                wu_ps[:, 0:128], wu_sb[:], wu_sb[:], start=(i == 0), stop=(i == 31)
            )

        # qw projection into per-head layout: head h=2mo on partitions
        # 0:64 of plane 2mo, h=2mo+1 on 64:128 of plane 2mo+1; the score
        # matmuls are row-tiled K=64 so the unwritten halves are never
        # read.
        for mo in range(4):
            ps = scpool.tile([128, 512], FP, tag="sc", name="qwps")
            for k in range(4):
                nc.tensor.matmul(
                    ps[:],
                    Wqw_t[:, k, mo * 128 : (mo + 1) * 128],
                    qT_sb[:, k, :],
                    start=(k == 0),
                    stop=(k == 3),
                )
            nc.vector.tensor_scalar_add(
                qw_pad[0:64, 2 * mo, :], ps[0:64, :], btiles["bqwT"][0:64, mo : mo + 1]
            )
            nc.vector.tensor_scalar_add(
                qw_pad[64:128, 2 * mo + 1, :],
                ps[64:128, :],
                btiles["bqwT"][64:128, mo : mo + 1],
            )
        kw_compute(0)
        vw_compute(0)
        stage_dma(1, engs=(nc.sync, nc.scalar, nc.sync, nc.scalar))
        Wvs_t = load_W("Wvs", eng=nc.sync)
        stage_dma(2)
        stage_dma(3)

        # --------------------- deferred sentence branch (runs at pidx 0-1)
        ews_all = apool.tile([128, 4, H, NB], FP, tag="ews_all")
        sden = apool.tile([128, 4, H], FP, tag="sden")
        srec = apool.tile([128, 4, H], FP, tag="srec")

        def sent_qs():
            for mo in range(4):
                ps = scpool.tile([128, 512], FP, tag="sc", name="qsps")
                for k in range(4):
                    nc.tensor.matmul(
                        ps[:],
                        Wqs_t[:, k, mo * 128 : (mo + 1) * 128],
                        qT_sb[:, k, :],
                        start=(k == 0),
                        stop=(k == 3),
                    )
                nc.vector.tensor_scalar_add(
                    qs_sb[:, mo, :], ps[:], btiles["bqsT"][:, mo : mo + 1]
                )

        def sent_ks():
            # ks projection: out [hdk(mo), nb]; b_ks cancels in the softmax.
            for mo in range(4):
                ps = scpool.tile([128, 512], FP, tag="sc", name="ksps")
                for k in range(4):
                    nc.tensor.matmul(
                        ps[:, 0:NB],
                        Wks_t[:, k, mo * 128 : (mo + 1) * 128],
                        ksT_sb[:, k, :],
                        start=(k == 0),
                        stop=(k == 3),
                    )
                nc.scalar.activation(ks_sb[:, mo, :], ps[:, 0:NB], ACTF.Copy)

        def sent_vs():
            # vs projection: out [nb, hdv]  (lhsT = vsT chunk, rhs = Wvs)
            ps = scpool.tile([128, 512], FP, tag="sc", name="vsps")
            for k in range(4):
                nc.tensor.matmul(
                    ps[0:NB, :],
                    vsT_sb[:, k, :],
                    Wvs_t[:, k, :],
                    start=(k == 0),
                    stop=(k == 3),
                )
            nc.scalar.activation(vs_sb[:, :], ps[0:NB, :], ACTF.Copy)

        def sent_attn(h0, h1):
            for h in range(h0, h1):
                hp, po = h // 2, (h % 2) * 64
                ps = scpool.tile([128, 512], FP, tag="sc", name="sattps")
                for qo in range(4):
                    nc.tensor.matmul(
                        ps[:, qo * NB : (qo + 1) * NB],
                        qs_sb[po : po + 64, hp, qo * 128 : (qo + 1) * 128],
                        ks_sb[po : po + 64, hp, :],
                        start=True,
                        stop=True,
                    )
                nc.scalar.activation(
                    ews_all[:, :, h, :],
                    ps[:, 0 : 4 * NB].rearrange("p (a x) -> p a x", x=NB),
                    ACTF.Exp,
                    scale=TOK_EXP_SCALE,
                )

        def sent_softmax():
            # batched sentence softmax: one reduce / recip / bcast-mul
            nc.vector.tensor_reduce(sden[:], ews_all[:], AX, ALU.add)
            nc.vector.reciprocal(srec[:], sden[:])
            rap = srec[:]
            rec_bc = bass.AP(
                rap.tensor, rap.offset, [list(x) for x in rap.ap] + [[0, NB]]
            )
            # on DVE, not gpsimd: this mult gates the first s3 factors and
            # gpsimd is ~2x slower per op on the startup critical path
            nc.vector.tensor_tensor(attn_sb[:], ews_all[:], rec_bc, ALU.mult)

        # per-pidx deferred startup work, sequenced to the DMA landing times;
        # the whole sentence-softmax chain goes at pidx 0 so attn_sb (which
        # gates every s3 factor) completes as early as possible
        startup_steps = {
            0: [sent_qs, sent_ks, lambda: sent_attn(0, 8), sent_softmax],
            2: [sent_vs],
        }

        # ---------------- phase 3+4+5: projections interleaved with attention
        tmppool_cm = tc.tile_pool(name="tmpp", bufs=4)
        tmppool = tmppool_cm.__enter__()

        # token scores + ctx_w accumulation, software-pipelined with skew-2
        # between scores/exp and the s3 consumption.
        NP = NBH // 2  # pairs per head

        def pidx_map(pidx):
            # four sweeps of 8 pidx: each head-pair covers block-pairs
            # {2s, 2s+1} in sweep s, so sweep one touches only staging group
            # sg0 and sg s is first read at pidx 8s -- the score pipeline
            # never waits on the kwT/vwT DMA trickle, and the projection
            # evictions spread evenly across the loop
            sweep, r = divmod(pidx, 8)
            return r // 2, sweep * 2 + (r % 2)

        ew_store = {}
        tmp_store = {}
        t1pool_cm = tc.tile_pool(name="tail1", bufs=1)
        t1pool = t1pool_cm.__enter__()
        aspool_cm = tc.tile_pool(name="asTring", bufs=2)
        aspool = aspool_cm.__enter__()
        Wfc1_sb = t1pool.tile([128, 4, D], BF, tag="Wfc1l")
        nc.gpsimd.dma_start(out=Wfc1_sb[:], in_=dt_in["Wfc1"][:])
        Wfc_sb = t1pool.tile([128, 8, D], BF, tag="Wfc")
        nc.gpsimd.dma_start(out=Wfc_sb[:], in_=dt_in["Wfc"][:])

        def sentence_tail(h):
            hp, po = h // 2, (h % 2) * 64
            asT_h = aspool.tile([NB, 4, 128], BF, tag="asT", name="asT_h")
            psT = s3pool.tile([128, 2, 512], FP, tag="s3", name="sattT")
            for qo in range(4):
                nc.tensor.transpose(
                    psT[0:NB, 0, qo * 128 : (qo + 1) * 128],
                    attn_sb[:, qo, h, :],
                    ident[:],
                )
            nc.vector.tensor_copy(asT_h[:, :, :], psT[0:NB, 0, :])
            # ctx_sT [dv(h), q] = vs.T @ attn_sT (+ b_vs per-partition)
            ps_t = s3pool.tile([128, 2, 512], FP, tag="s3", name="ctxsps")
            ps = ps_t[:, 0, :]
            nc.tensor.matmul(
                ps[0:64, :],
                vs_sb[:, h * 64 : (h + 1) * 64],
                asT_h[:, :, :],
                start=True,
                stop=True,
            )
            nc.vector.tensor_scalar_add(
                ctx_sT[po : po + 64, hp, :],
                ps[0:64, :],
                btiles["bvsT"][po : po + 64, hp : hp + 1],
            )
            if h == H - 1:
                # fc1: out [dm(mo), q] = Wfc1.T @ ctx_sT, x0.5 (+0.5*b_fc1)
                for mo in range(4):
                    ps_t = s3pool.tile([128, 2, 512], FP, tag="s3", name="fc1ps")
                    ps = ps_t[:, 0, :]
                    for k in range(4):
                        nc.tensor.matmul(
                            ps[:],
                            Wfc1_sb[:, k, mo * 128 : (mo + 1) * 128],
                            ctx_sT[:, k, :],
                            start=(k == 0),
                            stop=(k == 3),
                        )
                    nc.vector.tensor_scalar(
                        fc1T_sb[:, mo, :],
                        ps[:],
                        0.5,
                        btiles["bfc1T"][:, mo : mo + 1],
                        op0=ALU.mult,
                        op1=ALU.add,
                    )

        def emit_scores(pidx):
            hp, npair = pidx_map(pidx)
            tiles = []
            for g in range(2):
                n = npair * 2 + g
                for j in range(2):
                    tcg = 2 * n + j
                    # row-tiled pair: the two heads' K=64 matmuls occupy
                    # disjoint row-groups (partitions 0:64 / 64:128) and run
                    # concurrently in the PE array (~2x vs serial K=128).
                    ps_sc = scpool.tile([128, 2, 512], FP, tag="sc", name="scps")
                    nc.tensor.matmul(
                        ps_sc[:, 0, :],
                        kw_sb[0:64, hp, tcg * 128 : (tcg + 1) * 128],
                        qw_pad[0:64, 2 * hp, :],
                        start=True,
                        stop=True,
                    )
                    nc.tensor.matmul(
                        ps_sc[:, 1, :],
                        kw_sb[64:128, hp, tcg * 128 : (tcg + 1) * 128],
                        qw_pad[64:128, 2 * hp + 1, :],
                        start=True,
                        stop=True,
                    )
                    ew_t = ewpool.tile([128, 1024], BF, tag="ew", name="ew_t")
                    # Schraudolph fast-exp disabled: DVE is the busier engine
                    # in steady state, and dropping it buys back error margin
                    if False:
                        # Schraudolph fast-exp on DVE: bf16-as-int16 bit
                        # trick, exp(SCALE*x) ~= bitcast(i16(A*x + B)).
                        # Offloads 1/4 of the exp wall off the Scalar engine.
                        nc.vector.tensor_scalar(
                            ew_t[:].bitcast(mybir.dt.int16),
                            _flat2(ps_sc),
                            EXP_A,
                            EXP_B,
                            op0=ALU.mult,
                            op1=ALU.add,
                        )
                    else:
                        nc.scalar.activation(
                            ew_t[:], _flat2(ps_sc), ACTF.Exp, scale=TOK_EXP_SCALE
                        )
                    tiles.append(ew_t)  # order: (g, j)
            ew_store[pidx] = tiles

        def emit_s3_half(pidx, hs):
            hp, npair = pidx_map(pidx)
            tiles = ew_store[pidx] if hs == 0 else ew_store.pop(pidx)
            h = 2 * hp + hs
            s3t = s3pool.tile([128, 2, 512], FP, tag="s3", name="s3t")
            s3v = s3t.rearrange("p g (qo x) -> p g qo x", x=128)
            for g in range(2):
                n = npair * 2 + g
                for qo in range(4):
                    for j in range(2):
                        tcg = 2 * n + j
                        nc.tensor.matmul(
                            s3v[:, g, qo, 0 : DV + 1],
                            tiles[2 * g + j][
                                :, hs * 512 + qo * 128 : hs * 512 + (qo + 1) * 128
                            ],
                            vw_sb[:, tcg, h, :],
                            start=(j == 0),
                            stop=(j == 1),
                        )
            # factor = attn_s / denom for the 2x4 tiles of this head
            # (DVE has no tensor_tensor divide op on this target)
            rec_t = smpool.tile([128, 2, 4], FP, tag="rec_t")
            nc.vector.reciprocal(rec_t[:], s3v[:, :, :, DV])
            fac_t = smpool.tile([128, 2, 4], FP, tag="fac_t")
            nc.vector.tensor_mul(
                fac_t[:],
                rec_t[:],
                attn_sb[:, :, h, npair * 2 : npair * 2 + 2].rearrange(
                    "p qo g -> p g qo"
                ),
            )
            fap = fac_t[:]
            fac_bc = bass.AP(
                fap.tensor, fap.offset, [list(x) for x in fap.ap] + [[0, DV]]
            )
            tmp_t = tmppool.tile([128, 2, 4, DV], BF, tag="tmp", name="tmp_t")
            nc.vector.tensor_tensor(tmp_t[:], s3v[:, :, :, 0:DV], fac_bc, ALU.mult)
            # ctx accumulation on the otherwise-idle gpsimd engine
            acc_sl = ctx_acc[:, :, h * 64 : (h + 1) * 64]
            if npair == 0:
                nc.gpsimd.tensor_tensor(acc_sl, tmp_t[:, 0], tmp_t[:, 1], ALU.add)
            else:
                nc.gpsimd.tensor_tensor(acc_sl, acc_sl, tmp_t[:, 0], ALU.add)
                nc.gpsimd.tensor_tensor(acc_sl, acc_sl, tmp_t[:, 1], ALU.add)

        def emit_head_tail(hp):
            # b_vw is folded into vw_sb, so the tail is only the ctx_w
            # transpose for the finished head pair.
            dc = hp
            ps = s3pool.tile([128, 2, 512], FP, tag="s3", name="ctps")
            for qo in range(4):
                nc.tensor.transpose(
                    ps[:, 0, qo * 128 : (qo + 1) * 128],
                    ctx_acc[:, qo, dc * 128 : (dc + 1) * 128],
                    ident[:],
                )
            nc.vector.tensor_copy(ctx_wT[:, dc, :], ps[:, 0, :])

        fc_ps = []

        def emit_fc_part1():
            # final fc, cc chunks 0-6 (fc1T + ctx_wT dc0-2): all inputs ready
            # by pidx 31; only dc3 (cc7) must wait for the last head.
            for mp in range(2):
                ps = scpool.tile([128, 2, 512], FP, tag="sc", name="fcp1")
                fc_ps.append(ps)
                for half in range(2):
                    mo = mp * 2 + half
                    for cc in range(4):
                        nc.tensor.matmul(
                            ps[:, half, :],
                            Wfc_sb[:, cc, mo * 128 : (mo + 1) * 128],
                            fc1T_sb[:, cc, :],
                            start=(cc == 0),
                            stop=False,
                        )
                    for dc in range(3):
                        nc.tensor.matmul(
                            ps[:, half, :],
                            Wfc_sb[:, 4 + dc, mo * 128 : (mo + 1) * 128],
                            ctx_wT[:, dc, :],
                            start=False,
                            stop=False,
                        )

        HP = H // 2

        # sg1-3 projections spread across the loop (the two-sweep pidx_map
        # defers the first sg2/sg3 use to pidx 16), so their PE matmuls and
        # DVE evictions don't pile up inside head-pair 0
        proj_sched = {
            4: lambda: kw_compute(1),
            6: lambda: vw_compute(1),
            10: lambda: kw_compute(2),
            12: lambda: vw_compute(2),
            18: lambda: kw_compute(3),
            20: lambda: vw_compute(3),
        }

        for pidx in range(HP * NP + 4):
            # priority order inside an iteration: s3 first (frees ew-ring
            # slots the exps are waiting on), then scores, then the
            # deadline-free projections
            if pidx >= 2 and pidx - 2 < HP * NP:
                emit_s3_half(pidx - 2, 0)
                emit_s3_half(pidx - 2, 1)
            if pidx in proj_sched:
                proj_sched[pidx]()
            # startup steps BEFORE the scores: the sentence exps then rank
            # ahead of the token exps on ACT and run in its pre-23us idle
            # window, so attn_sb (which gates the whole s3/fac chain and,
            # through the s3 PSUM ring, the exp stream) lands ~5us earlier
            for st in startup_steps.get(pidx, ()):
                st()
            if pidx < HP * NP:
                emit_scores(pidx)
            # sentence tails early (PE has slack under the exp pacing);
            # fc1 lands at pidx 20, so only the last head transposes + 4 fc
            # matmuls trail the loop.
            if 7 <= pidx <= 21 and pidx % 2 == 1:
                sentence_tail((pidx - 7) // 2)
            if pidx >= 28 and (pidx - 28) % 2 == 0:
                emit_head_tail((pidx - 28) // 2)
        # fc_part1 after the loop: its scpool tiles persist into phase 6, and
        # its 28 matmuls must rank below the final s3/tail chain so they fill
        # PE idle time instead of delaying the tail
        emit_fc_part1()

        aspool_cm.__exit__(None, None, None)
        t1pool_cm.__exit__(None, None, None)
        tmppool_cm.__exit__(None, None, None)
        apool_cm.__exit__(None, None, None)
        stpool_cm.__exit__(None, None, None)

        # ---------------- phase 6: sentence-ctx tail, then final fc
        lpool_cm = tc.tile_pool(name="late", bufs=1)
        lpool = lpool_cm.__enter__()
        outT_sb = lpool.tile([128, 4, LQ], BF, tag="outT_sb")
        # all 4 dc3 matmuls back-to-back (distinct PSUM banks), then the
        # evictions + output DMAs, so the tail chain isn't MM/evict ping-pong
        for mo in range(4):
            nc.tensor.matmul(
                fc_ps[mo // 2][:, mo % 2, :],
                Wfc_sb[:, 7, mo * 128 : (mo + 1) * 128],
                ctx_wT[:, 3, :],
                start=False,
                stop=True,
            )
        for mo in range(4):
            # split the four evictions between DVE and the (now idle) Scalar
            # engine so the tail chain isn't serialized on one engine
            if mo % 2 == 0:
                nc.vector.tensor_scalar_add(
                    outT_sb[:, mo, :], fc_ps[mo // 2][:, mo % 2, :],
                    btiles["bfcT"][:, mo : mo + 1],
                )
            else:
                nc.scalar.activation(
                    outT_sb[:, mo, :], fc_ps[mo // 2][:, mo % 2, :],
                    ACTF.Identity, bias=btiles["bfcT"][:, mo : mo + 1],
                )
            (nc.sync if mo % 2 == 0 else nc.scalar).dma_start(
                out=outT_d[mo * 128 : (mo + 1) * 128, :], in_=outT_sb[:, mo, :]
            )

        lpool_cm.__exit__(None, None, None)
        smpool_cm.__exit__(None, None, None)
        ewpool_cm.__exit__(None, None, None)
        s3pool_cm.__exit__(None, None, None)
        scpool_cm.__exit__(None, None, None)
        ppool_cm.__exit__(None, None, None)

    if not for_sim:
        ns = _split_multi_waits(nc)
        print(f"[kernel] split {ns} extra sem waits onto NOPs", file=sys.stderr)
    return nc


_NC_CACHE = None


def _get_nc():
    global _NC_CACHE
    if _NC_CACHE is None:
        _NC_CACHE = build_program()
    return _NC_CACHE


def make_in_maps(inputs):
    f = lambda x: np.ascontiguousarray(np.asarray(x, dtype=np.float32))
    q, k_w, v_w, k_s, v_s = (f(inputs[n]) for n in ["q", "k_w", "v_w", "k_s", "v_s"])
    W = {n: f(inputs[n]) for n in inputs if n.startswith(("W_", "b_"))}

    def bT(v, scale=1.0):
        return np.ascontiguousarray((v * scale).reshape(4, 128).T)

    def tl(a, kc=4):
        return np.ascontiguousarray(a.reshape(kc, 128, a.shape[-1]).transpose(1, 0, 2))

    bf = ml_dtypes.bfloat16
    f8 = ml_dtypes.float8_e4m3
    # fp8 weights scaled x16 so std~0.02 entries stay in e4m3's normal range
    WSC = 16.0
    shared = {
        "Wqs": tl((W["W_qs"] * WSC).astype(f8)),
        "Wks": tl((W["W_ks"] * WSC).astype(f8)),
        "Wvs": tl(W["W_vs"].astype(bf)),
        "Wqw": tl((W["W_qw"] * WSC).astype(f8)),
        "Wkw": tl((W["W_kw"] * WSC).astype(f8)),
        "Wvw": tl((W["W_vw"] * WSC).astype(f8)),
        "Wfc1": tl(W["W_fc1"].astype(bf)), "Wfc": tl(W["W_fc"].astype(bf), kc=8),
        "bqsT": bT(W["b_qs"], WSC), "bksT": bT(W["b_ks"]),
        "bqwT": bT(W["b_qw"], WSC),
        "bkwT": bT(W["b_kw"]), "bvsT": bT(W["b_vs"]),
        "bfc1T": bT(W["b_fc1"], 0.5), "bfcT": bT(W["b_fc"], 0.5),
        "bvw": W["b_vw"] * WSC,
    }
    in_maps = []
    for c in range(N_CORES):
        b, half = divmod(c, 2)
        blk = slice(half * NBH, half * NBH + NBH)
        ks_r = np.roll(k_s[b], -half * NBH, axis=0)
        vs_r = np.roll(v_s[b], -half * NBH, axis=0)
        m = dict(shared)
        m["qT"] = tl(q[b].T.astype(bf))
        kwT = k_w[b, blk].reshape(NTOK, D).T.astype(f8)
        vwT = v_w[b, blk].reshape(NTOK, D).T.astype(f8)

        def stg(a):
            a4 = a.reshape(4, 128, 4, 1024)            # [k, p, sg, t]
            return np.ascontiguousarray(a4.transpose(2, 1, 0, 3).reshape(4, 128, NTOK))

        m["kwT"] = stg(kwT)
        m["vwT"] = stg(vwT)
        m["ksT"] = tl(ks_r.T.astype(bf))
        m["vsT"] = tl(vs_r.T.astype(bf))
        in_maps.append(m)
    return in_maps


def run_cores(inputs, trace=False):
    nc = _get_nc()
    in_maps = make_in_maps(inputs)
    res = run_bass_kernel_spmd(nc, in_maps, list(range(N_CORES)), trace=trace)
    return res


def assemble(res):
    out = np.empty((B, LQ, D), dtype=np.float32)
    for b in range(B):
        out[b] = (
            res.results[2 * b]["outT"].astype(np.float32)
            + res.results[2 * b + 1]["outT"].astype(np.float32)
        ).T
    return out


def kernel(**inputs) -> np.ndarray:
    res = run_cores(inputs, trace=False)
    return assemble(res)


if __name__ == "__main__":
    import reference

    inp = {k: np.asarray(v) for k, v in reference.setup_inputs().items()}
    out = kernel(**inp)
    exp = np.asarray(reference.reference(**inp))
    err = np.abs(out - exp).max() / np.abs(exp).max()
    print("max rel err:", err)



# revision 85
# speedup vs baseline: 1.0041x; 1.0041x over previous
"""
Trainium2 Bass kernel for nn_MultiHeadHierarchicalAttention.

Sharding: 8 cores = (batch b in 0..3) x (block-half in 0..1).
Each core handles one batch and 16 of the 32 blocks for the token-level
attention; the (small) sentence-level attention + fc1 branch is computed
redundantly on both cores of a batch, scaled by 0.5, and the host sums the
two per-batch partial outputs (the final fc is linear, so partial ctx_w
contributions simply add).

Pipeline: (head-pair, block-pair) main loop in TWO SWEEPS (each head-pair
covers block-pairs 0-3 in sweep one, 4-7 in sweep two) so the sg2/sg3
kw/vw projections spread across the loop middle instead of piling into
head-pair 0; skew-2 scores->exp->s3; Scalar engine reserved for exp (the
~1.07us/tile exp stream is the kernel's wall); ctx accumulation on GpSimd;
deferred sentence branch overlapping the startup DMA trickle; final fc
emitted after the loop so it ranks below the tail chain.

Key tricks:
  - score matmuls are K=64 row-tiled PAIRS (tile_position 0/64) running
    concurrently in the PE array (~2x vs serial K=128 with padded qw).
  - fp8 weights (W_kw/W_vw/W_qw/W_qs/W_ks) are x16-scaled on the host so
    std-0.02 entries stay in e4m3's normal range; the 1/256 is folded into
    the exp scale and the vw ones-column is 16.0 so the softmax factor
    self-corrects. Mixed fp8-lhsT x bf16-rhs matmuls are used directly.
  - startup DMAs ride the sync+scalar hardware rings (FIFO = priority,
    ~215 GB/s pooled) with few triggers (~0.7us each); small bias loads go
    before bulky weights on gpsimd's ~85 GB/s software queue.

Device layouts (per core, partition dim first):
  qT   [D, LQ]      kwT/vwT [D, 4096]   ksT/vsT [D, 32] (block-rolled)
  projections keep features on partitions (kw) or tokens on partitions (vw)
  token scores are computed as [t, q] tiles (K=dk=64), exp'd on ACT into
  bf16, and consumed per block by S3 matmuls producing [q, dv+1] partials
  (the +1 "ones" column of vw gives the softmax denominator for free).
  The sentence-attention factor attn_s/denom scales the per-block ctx on
  DVE; b_vw is folded into vw_sb so no separate bias pass is needed.
  ctx_w is PE-transposed at the end and fused into the final fc, which is
  emitted as outT [D, LQ] (host transposes and sums core pairs).
"""

import sys

sys.path.insert(0, "/opt/trn_rl_repo")

import numpy as np
import ml_dtypes
import concourse.bass as bass
import concourse.tile as tile
from concourse import mybir
from concourse.bass_utils import run_bass_kernel_spmd
from concourse.vector_clock import ScopedClock
from concourse.masks import make_identity

# ---------------------------------------------------------------- constants
B, LQ, NB, NT = 4, 512, 32, 256
D, H, DK, DV = 512, 8, 64, 64
NBH = NB // 2  # blocks per core
NTOK = NBH * NT  # tokens per core = 4096
NTC = NTOK // 128  # 32 token chunks of 128
SCALE = 0.125
# fp8 weights are scaled x16 on the host so W std ~0.02 lands in e4m3's
# normal range (raw 0.02-std weights round through subnormals at huge
# relative error). The projections then produce 16x-scaled kw/qw/qs/ks;
# evictions and exp scales fold the 1/16 back in.
WSC = 16.0
TOK_EXP_SCALE = SCALE / (WSC * WSC)  # qw and kw both carry x16
# Schraudolph fast-exp (bf16-as-int16): exp(s*x) ~= bitcast(i16(A*x+B))
EXP_A = 128 * 1.4426950408889634 * TOK_EXP_SCALE
EXP_B = 16250.5
FP = mybir.dt.float32
FR = mybir.dt.float32r
BF = mybir.dt.bfloat16
F8 = mybir.dt.float8e4
N_CORES = 8

AX = mybir.AxisListType.X
ALU = mybir.AluOpType
ACTF = mybir.ActivationFunctionType


# --------------------------------------------------------- drain workaround
def _patched_drain_and_barrier(self, tick_clock, wait_clock):
    # walrus in this container rejects >1 sem wait on a single TPB_CTRL
    # instruction ("Too many sync wait commands"); split the kernel-tail
    # drain waits across one-wait NOPs.
    nop_inst = self.nc.sync.nop(nofuse=True)
    wait_clock.add_sem_waits(nop_inst.ins, ScopedClock({None: tick_clock.global_clock}))
    waits = list(nop_inst.ins.sync_info.on_wait or [])
    if len(waits) > 1:
        nop_inst.ins.sync_info.on_wait = waits[:1]
        rest = waits[1:]
        while rest:
            extra = self.nc.sync.nop(nofuse=True)
            if extra.ins.sync_info is None:
                extra.ins.sync_info = mybir.SyncInfo(on_wait=[], on_update=[])
            extra.ins.sync_info.on_wait = rest[:1]
            rest = rest[1:]
    self.nc.sync.drain()
    self.nc.all_engine_barrier()
    assert self.sems is not None
    popped = self.nc._tile_sem_poison_stack.pop()
    assert popped is self._sem_poison
    self.nc.clear_and_free_semaphores(list(self.sems.allocated().values()))
    self.nc.all_engine_barrier()


_ORIG_DRAIN_AND_BARRIER = tile.TileContext._drain_and_barrier
tile.TileContext._drain_and_barrier = _patched_drain_and_barrier


def _r(ap):
    """View an f32 AP as float32r so matmuls run at 1 cycle/row."""
    return ap.bitcast(mybir.dt.float32r)


_NO_SPLIT_OPCODES = {
    "CollectiveCompute",
    "EventSemaphore",
}
_split_counter = [0]


def _split_multi_waits(nc):
    """This container's walrus accepts at most ONE sem wait per TPB
    instruction; hoist extra waits onto same-engine NOPs placed before."""
    n_split = 0
    for fn in nc.m.functions:
        for bb in fn.blocks:
            changed = False
            out = []
            for inst in bb.instructions:
                si = inst.sync_info
                if (
                    si is not None
                    and si.on_wait
                    and len(list(si.on_wait)) > 1
                    and inst.opcode not in _NO_SPLIT_OPCODES
                ):
                    waits = list(si.on_wait)
                    for w in waits[:-1]:
                        _split_counter[0] += 1
                        nop = mybir.InstNoOp(name=f"I-wsplit-{_split_counter[0]}")
                        nop.engine = inst.engine
                        nop.sync_info = mybir.SyncInfo(on_wait=[w], on_update=[])
                        out.append(nop)
                        n_split += 1
                    si.on_wait = waits[-1:]
                    changed = True
                out.append(inst)
            if changed:
                bb.instructions = out
    return n_split


def _flat2(ap):
    """[p, a, b] -> [p, a*b]"""
    return ap.rearrange("p a b -> p (a b)")


# ------------------------------------------------------------ program build
def build_program(for_sim=False):
    # the walrus-only wait-splitting workarounds confuse CoreSim's race
    # detector; skip them when building for simulation.
    tile.TileContext._drain_and_barrier = (
        _ORIG_DRAIN_AND_BARRIER if for_sim else _patched_drain_and_barrier
    )
    nc = bass.Bass("TRN2", target_bir_lowering=False, debug=False, num_devices=N_CORES)

    dt_in = {}
    for name, shape in [
        ("kwT", [4, 128, NTOK]),
        ("vwT", [4, 128, NTOK]),
        ("Wkw", [128, 4, H * DK]),
        ("Wvw", [128, 4, H * DV]),
        ("Wqs", [128, 4, H * DK]),
        ("Wks", [128, 4, H * DK]),
        ("Wqw", [128, 4, H * DK]),
    ]:
        dt_in[name] = nc.dram_tensor(name, shape, F8, kind="ExternalInput").ap()
    for name, shape in [
        ("qT", [128, 4, LQ]),
        ("ksT", [128, 4, NB]),
        ("vsT", [128, 4, NB]),
        ("Wvs", [128, 4, H * DV]),
        ("Wfc", [128, 8, D]),
        ("Wfc1", [128, 4, D]),
    ]:
        dt_in[name] = nc.dram_tensor(name, shape, BF, kind="ExternalInput").ap()
    for name, shape in [
        ("bqsT", [128, 4]),
        ("bksT", [128, 4]),
        ("bqwT", [128, 4]),
        ("bkwT", [128, 4]),
        ("bvsT", [128, 4]),
        ("bfc1T", [128, 4]),
        ("bfcT", [128, 4]),
        ("bvw", [H * DV]),
    ]:
        dt_in[name] = nc.dram_tensor(name, shape, FP, kind="ExternalInput").ap()
    outT_d = nc.dram_tensor("outT", [D, LQ], BF, kind="ExternalOutput").ap()

    with tile.TileContext(nc) as tc:
        # ------------------------------------------------ persistent pools
        ppool_cm = tc.tile_pool(name="persist", bufs=1)
        ppool = ppool_cm.__enter__()
        scpool_cm = tc.tile_pool(name="scps", bufs=2, space="PSUM")
        scpool = scpool_cm.__enter__()
        s3pool_cm = tc.tile_pool(name="s3ps", bufs=2, space="PSUM")
        s3pool = s3pool_cm.__enter__()
        ewpool_cm = tc.tile_pool(name="ewp", bufs=18)
        ewpool = ewpool_cm.__enter__()
        smpool_cm = tc.tile_pool(name="small", bufs=8)
        smpool = smpool_cm.__enter__()

        ident = ppool.tile([128, 128], FP, tag="ident")

        # persistent sbuf tensors
        qw_pad = ppool.tile([128, H, LQ], BF, tag="qw_pad")
        ks_sb = ppool.tile([128, 4, NB], BF, tag="ks_sb")
        attn_sb = ppool.tile([128, 4, H, NB], FP, tag="attn_sb")
        fc1T_sb = ppool.tile([128, 4, LQ], BF, tag="fc1T")
        kw_sb = ppool.tile([128, 4, NTOK], BF, tag="kw_sb")
        vw_sb = ppool.tile([128, NTC, H, DV + 1], BF, tag="vw_sb")
        ctx_acc = ppool.tile([128, 4, H * DV], FP, tag="ctx_acc")
        vs_sb = ppool.tile([NB, H * DV], BF, tag="vs_sb")
        ctx_sT = ppool.tile([128, 4, LQ], BF, tag="ctx_sT")

        # staging pool + kw/vw weight tiles created early so the big DMAs
        # stream during the small branch.
        stpool_cm = tc.tile_pool(name="stage", bufs=3)
        stpool = stpool_cm.__enter__()
        Wkw_sb = ppool.tile([128, 4, H * DK], F8, tag="Wkw")
        Wvw_sb = ppool.tile([128, 4, H * DV], F8, tag="Wvw")
        ctx_wT = ppool.tile([128, 4, LQ], BF, tag="ctx_wT")
        stg_store = {}

        def dma_halves(tile4, in_ap, eng0, eng1):
            # split a [128, 4, n] load across two DMA queues: one queue tops
            # out around ~70 GB/s, which serializes the startup badly.
            eng0.dma_start(out=tile4[:, 0:2, :], in_=in_ap[:, 0:2, :])
            eng1.dma_start(out=tile4[:, 2:4, :], in_=in_ap[:, 2:4, :])

        def stage_dma(sg, engs=None):
            kstg = stpool.tile([128, 4, 1024], F8, tag="kstg", name="kstg")
            vstg = stpool.tile([128, 4, 1024], F8, tag="vstg", name="vstg")
            if engs is None:
                nc.sync.dma_start(
                    out=kstg.rearrange("p a b -> p (a b)"), in_=dt_in["kwT"][sg]
                )
                nc.sync.dma_start(
                    out=vstg.rearrange("p a b -> p (a b)"), in_=dt_in["vwT"][sg]
                )
            else:
                ka = dt_in["kwT"][sg].rearrange("p (a b) -> p a b", b=1024)
                va = dt_in["vwT"][sg].rearrange("p (a b) -> p a b", b=1024)
                dma_halves(kstg, ka, engs[0], engs[1])
                dma_halves(vstg, va, engs[2], engs[3])
            stg_store[sg] = (kstg, vstg)

        DR = mybir.MatmulPerfMode.DoubleRow

        def kw_compute(sg):
            stg = stg_store[sg][0]
            for mo in range(4):
                ps = s3pool.tile([128, 2, 512], FP, tag="s3", name="kwps")
                for j in range(2):
                    for s in range(2):
                        nc.tensor.matmul(
                            ps[:, j, :],
                            Wkw_sb[:, 2 * s : 2 * s + 2, mo * 128 : (mo + 1) * 128],
                            stg[:, 2 * s : 2 * s + 2, j * 512 : (j + 1) * 512],
                            start=(s == 0),
                            stop=(s == 1),
                            perf_mode=DR,
                        )
                # NOTE: b_kw adds a per-(h,q) constant to every score in every
                # block; it cancels in the within-block softmax and in the
                # attn_s/denom ratio, so it is dropped entirely.
                if sg == 0:
                    # startup: ACT is idle until the first exp (~25us), so
                    # the sg0 evictions run there, off the DVE critical
                    # path that gates the first score matmuls
                    nc.scalar.activation(
                        kw_sb[:, mo, sg * 1024 : (sg + 1) * 1024],
                        _flat2(ps),
                        ACTF.Copy,
                    )
                else:
                    nc.vector.tensor_copy(
                        kw_sb[:, mo, sg * 1024 : (sg + 1) * 1024], _flat2(ps)
                    )

        def vw_compute(sg):
            stg = stg_store[sg][1]
            for tp in range(4):
                ps = s3pool.tile([128, 2, 512], FP, tag="s3", name="vwps")
                for j in range(2):
                    tcl = tp * 2 + j
                    for s in range(2):
                        nc.tensor.matmul(
                            ps[:, j, :],
                            stg[:, 2 * s : 2 * s + 2, tcl * 128 : (tcl + 1) * 128],
                            Wvw_sb[:, 2 * s : 2 * s + 2, :],
                            start=(s == 0),
                            stop=(s == 1),
                            perf_mode=DR,
                        )
                tc0 = sg * 8 + tp * 2
                # fold b_vw into vw: the per-block softmax-weighted sum of
                # (vw + b) equals ctx + denom*b, and the attn_s/denom factor
                # turns that into exactly the b_vw bias term summed over
                # blocks — no separate head-tail bias pass needed.
                bap = bvw_bc[:].rearrange("p (h x) -> p h x", x=DV)
                bvw_view = bass.AP(
                    bap.tensor,
                    bap.offset,
                    [list(bap.ap[0])] + [[0, 2]] + [list(d) for d in bap.ap[1:]],
                )
                nc.vector.tensor_tensor(
                    vw_sb[:, tc0 : tc0 + 2, :, 0:DV],
                    ps.rearrange("p a (h x) -> p a h x", x=DV),
                    bvw_view,
                    ALU.add,
                )

        # ------------------------------------------- phase 1: token-path start
        # Critical path: qT+Wqw -> qw proj -> kw sg0 -> first scores -> exp.
        # The sentence branch is deferred into the first two main-loop
        # iterations so ACT starts exp'ing as early as possible.
        apool_cm = tc.tile_pool(name="phaseA", bufs=1)
        apool = apool_cm.__enter__()

        qT_sb = apool.tile([128, 4, LQ], BF, tag="qT_sb")
        ksT_sb = apool.tile([128, 4, NB], BF, tag="ksT_sb")
        vsT_sb = apool.tile([128, 4, NB], BF, tag="vsT_sb")
        qs_sb = apool.tile([128, 4, LQ], BF, tag="qs_sb")

        def load_W(wn, wdt=BF, eng=None, engs=None):
            t = apool.tile([128, 4, D], wdt, tag=f"W_{wn}", name=wn)
            if engs is not None:
                dma_halves(t, dt_in[wn], engs[0], engs[1])
            else:
                (eng or nc.gpsimd).dma_start(out=t[:], in_=dt_in[wn][:])
            return t

        # Startup DMAs. Each dma_start trigger costs ~0.7us on the issuing
        # engine, and transfers stripe across the 16 DMA engines regardless,
        # so: few triggers, hardware ring (sync) drains ~215 GB/s FIFO in
        # emission order = priority, gpsimd's software queue (~85 GB/s) runs
        # the fp8 weights + small loads concurrently. Nothing on scalar --
        # the exp wall lives there.
        # the scalar engine is idle until the first exp (~20us), so its DMA
        # ring carries half of the startup-critical loads for free
        dma_halves(qT_sb, dt_in["qT"], nc.sync, nc.scalar)
        Wqw_t = load_W("Wqw", F8)
        # tiny loads (biases feed the qw/qs evictions!) must not queue behind
        # the bulky fp8 weights on gpsimd's slow software queue
        btiles = {}
        for bn in ["bqsT", "bqwT", "bvsT", "bfc1T", "bfcT"]:
            t = ppool.tile([128, 4], FP, tag=bn)
            nc.gpsimd.dma_start(out=t[:], in_=dt_in[bn][:])
            btiles[bn] = t
        bvw_bc = ppool.tile([128, H * DV], FP, tag="bvw_bc")
        src = dt_in["bvw"]
        bcast_ap = bass.AP(src.tensor, src.offset, [[0, 128]] + [list(x) for x in src.ap])
        nc.gpsimd.dma_start(out=bvw_bc[:], in_=bcast_ap)
        nc.gpsimd.dma_start(out=ksT_sb[:], in_=dt_in["ksT"][:])
        nc.gpsimd.dma_start(out=vsT_sb[:], in_=dt_in["vsT"][:])
        nc.sync.dma_start(out=Wkw_sb[:], in_=dt_in["Wkw"][:])
        stage_dma(0, engs=(nc.sync, nc.scalar, nc.sync, nc.scalar))
        nc.sync.dma_start(out=Wvw_sb[:], in_=dt_in["Wvw"][:])
        Wqs_t = load_W("Wqs", F8, eng=nc.sync)
        Wks_t = load_W("Wks", F8, eng=nc.sync)
        make_identity(nc, ident)
        nc.vector.memset(vw_sb[:, :, :, DV : DV + 1], WSC)
        # PE warm-up: ~4us of dummy matmuls while the startup DMAs stream,
        # so the HAM clock-gate reaches 2.4 GHz before the real projections
        # (otherwise qw/kw0 run at the cold 1.2 GHz rate)
        wu_sb = apool.tile([128, 128], BF, tag="warmup")
        nc.vector.memset(wu_sb[:], 0.0)
        wu_ps = scpool.tile([128, 512], FP, tag="sc", name="wups")
        for i in range(32):
            nc.tensor.matmul(
                wu_ps[:, 0:128], wu_sb[:], wu_sb[:], start=(i == 0), stop=(i == 31)
            )

        # qw projection into per-head layout: head h=2mo on partitions
        # 0:64 of plane 2mo, h=2mo+1 on 64:128 of plane 2mo+1; the score
        # matmuls are row-tiled K=64 so the unwritten halves are never
        # read.
        for mo in range(4):
            ps = scpool.tile([128, 512], FP, tag="sc", name="qwps")
            for k in range(4):
                nc.tensor.matmul(
                    ps[:],
                    Wqw_t[:, k, mo * 128 : (mo + 1) * 128],
                    qT_sb[:, k, :],
                    start=(k == 0),
                    stop=(k == 3),
                )
            nc.vector.tensor_scalar_add(
                qw_pad[0:64, 2 * mo, :], ps[0:64, :], btiles["bqwT"][0:64, mo : mo + 1]
            )
            nc.vector.tensor_scalar_add(
                qw_pad[64:128, 2 * mo + 1, :],
                ps[64:128, :],
                btiles["bqwT"][64:128, mo : mo + 1],
            )
        kw_compute(0)
        vw_compute(0)
        stage_dma(1, engs=(nc.sync, nc.scalar, nc.sync, nc.scalar))
        Wvs_t = load_W("Wvs", eng=nc.sync)
        stage_dma(2)
        stage_dma(3)

        # --------------------- deferred sentence branch (runs at pidx 0-1)
        ews_all = apool.tile([128, 4, H, NB], FP, tag="ews_all")
        sden = apool.tile([128, 4, H], FP, tag="sden")
        srec = apool.tile([128, 4, H], FP, tag="srec")

        def sent_qs():
            for mo in range(4):
                ps = scpool.tile([128, 512], FP, tag="sc", name="qsps")
                for k in range(4):
                    nc.tensor.matmul(
                        ps[:],
                        Wqs_t[:, k, mo * 128 : (mo + 1) * 128],
                        qT_sb[:, k, :],
                        start=(k == 0),
                        stop=(k == 3),
                    )
                nc.vector.tensor_scalar_add(
                    qs_sb[:, mo, :], ps[:], btiles["bqsT"][:, mo : mo + 1]
                )

        def sent_ks():
            # ks projection: out [hdk(mo), nb]; b_ks cancels in the softmax.
            for mo in range(4):
                ps = scpool.tile([128, 512], FP, tag="sc", name="ksps")
                for k in range(4):
                    nc.tensor.matmul(
                        ps[:, 0:NB],
                        Wks_t[:, k, mo * 128 : (mo + 1) * 128],
                        ksT_sb[:, k, :],
                        start=(k == 0),
                        stop=(k == 3),
                    )
                nc.scalar.activation(ks_sb[:, mo, :], ps[:, 0:NB], ACTF.Copy)

        def sent_vs():
            # vs projection: out [nb, hdv]  (lhsT = vsT chunk, rhs = Wvs)
            ps = scpool.tile([128, 512], FP, tag="sc", name="vsps")
            for k in range(4):
                nc.tensor.matmul(
                    ps[0:NB, :],
                    vsT_sb[:, k, :],
                    Wvs_t[:, k, :],
                    start=(k == 0),
                    stop=(k == 3),
                )
            nc.scalar.activation(vs_sb[:, :], ps[0:NB, :], ACTF.Copy)

        def sent_attn(h0, h1):
            for h in range(h0, h1):
                hp, po = h // 2, (h % 2) * 64
                ps = scpool.tile([128, 512], FP, tag="sc", name="sattps")
                for qo in range(4):
                    nc.tensor.matmul(
                        ps[:, qo * NB : (qo + 1) * NB],
                        qs_sb[po : po + 64, hp, qo * 128 : (qo + 1) * 128],
                        ks_sb[po : po + 64, hp, :],
                        start=True,
                        stop=True,
                    )
                nc.scalar.activation(
                    ews_all[:, :, h, :],
                    ps[:, 0 : 4 * NB].rearrange("p (a x) -> p a x", x=NB),
                    ACTF.Exp,
                    scale=TOK_EXP_SCALE,
                )

        def sent_softmax():
            # batched sentence softmax: one reduce / recip / bcast-mul
            nc.vector.tensor_reduce(sden[:], ews_all[:], AX, ALU.add)
            nc.vector.reciprocal(srec[:], sden[:])
            rap = srec[:]
            rec_bc = bass.AP(
                rap.tensor, rap.offset, [list(x) for x in rap.ap] + [[0, NB]]
            )
            # on DVE, not gpsimd: this mult gates the first s3 factors and
            # gpsimd is ~2x slower per op on the startup critical path
            nc.vector.tensor_tensor(attn_sb[:], ews_all[:], rec_bc, ALU.mult)

        # per-pidx deferred startup work, sequenced to the DMA landing times;
        # the whole sentence-softmax chain goes at pidx 0 so attn_sb (which
        # gates every s3 factor) completes as early as possible
        startup_steps = {
            0: [sent_qs, sent_ks, lambda: sent_attn(0, 8), sent_softmax],
            2: [sent_vs],
        }

        # ---------------- phase 3+4+5: projections interleaved with attention
        tmppool_cm = tc.tile_pool(name="tmpp", bufs=4)
        tmppool = tmppool_cm.__enter__()

        # token scores + ctx_w accumulation, software-pipelined with skew-2
        # between scores/exp and the s3 consumption.
        NP = NBH // 2  # pairs per head

        def pidx_map(pidx):
            # four sweeps of 8 pidx: each head-pair covers block-pairs
            # {2s, 2s+1} in sweep s, so sweep one touches only staging group
            # sg0 and sg s is first read at pidx 8s -- the score pipeline
            # never waits on the kwT/vwT DMA trickle, and the projection
            # evictions spread evenly across the loop
            sweep, r = divmod(pidx, 8)
            return r // 2, sweep * 2 + (r % 2)

        ew_store = {}
        tmp_store = {}
        t1pool_cm = tc.tile_pool(name="tail1", bufs=1)
        t1pool = t1pool_cm.__enter__()
        aspool_cm = tc.tile_pool(name="asTring", bufs=2)
        aspool = aspool_cm.__enter__()
        Wfc1_sb = t1pool.tile([128, 4, D], BF, tag="Wfc1l")
        nc.gpsimd.dma_start(out=Wfc1_sb[:], in_=dt_in["Wfc1"][:])
        Wfc_sb = t1pool.tile([128, 8, D], BF, tag="Wfc")
        nc.gpsimd.dma_start(out=Wfc_sb[:], in_=dt_in["Wfc"][:])

        def sentence_tail(h):
            hp, po = h // 2, (h % 2) * 64
            asT_h = aspool.tile([NB, 4, 128], BF, tag="asT", name="asT_h")
            psT = s3pool.tile([128, 2, 512], FP, tag="s3", name="sattT")
            for qo in range(4):
                nc.tensor.transpose(
                    psT[0:NB, 0, qo * 128 : (qo + 1) * 128],
                    attn_sb[:, qo, h, :],
                    ident[:],
                )
            nc.vector.tensor_copy(asT_h[:, :, :], psT[0:NB, 0, :])
            # ctx_sT [dv(h), q] = vs.T @ attn_sT (+ b_vs per-partition)
            ps_t = s3pool.tile([128, 2, 512], FP, tag="s3", name="ctxsps")
            ps = ps_t[:, 0, :]
            nc.tensor.matmul(
                ps[0:64, :],
                vs_sb[:, h * 64 : (h + 1) * 64],
                asT_h[:, :, :],
                start=True,
                stop=True,
            )
            nc.vector.tensor_scalar_add(
                ctx_sT[po : po + 64, hp, :],
                ps[0:64, :],
                btiles["bvsT"][po : po + 64, hp : hp + 1],
            )
            if h == H - 1:
                # fc1: out [dm(mo), q] = Wfc1.T @ ctx_sT, x0.5 (+0.5*b_fc1)
                for mo in range(4):
                    ps_t = s3pool.tile([128, 2, 512], FP, tag="s3", name="fc1ps")
                    ps = ps_t[:, 0, :]
                    for k in range(4):
                        nc.tensor.matmul(
                            ps[:],
                            Wfc1_sb[:, k, mo * 128 : (mo + 1) * 128],
                            ctx_sT[:, k, :],
                            start=(k == 0),
                            stop=(k == 3),
                        )
                    nc.vector.tensor_scalar(
                        fc1T_sb[:, mo, :],
                        ps[:],
                        0.5,
                        btiles["bfc1T"][:, mo : mo + 1],
                        op0=ALU.mult,
                        op1=ALU.add,
                    )

        def emit_scores(pidx):
            hp, npair = pidx_map(pidx)
            tiles = []
            for g in range(2):
                n = npair * 2 + g
                for j in range(2):
                    tcg = 2 * n + j
                    # row-tiled pair: the two heads' K=64 matmuls occupy
                    # disjoint row-groups (partitions 0:64 / 64:128) and run
                    # concurrently in the PE array (~2x vs serial K=128).
                    ps_sc = scpool.tile([128, 2, 512], FP, tag="sc", name="scps")
                    nc.tensor.matmul(
                        ps_sc[:, 0, :],
                        kw_sb[0:64, hp, tcg * 128 : (tcg + 1) * 128],
                        qw_pad[0:64, 2 * hp, :],
                        start=True,
                        stop=True,
                    )
                    nc.tensor.matmul(
                        ps_sc[:, 1, :],
                        kw_sb[64:128, hp, tcg * 128 : (tcg + 1) * 128],
                        qw_pad[64:128, 2 * hp + 1, :],
                        start=True,
                        stop=True,
                    )
                    ew_t = ewpool.tile([128, 1024], BF, tag="ew", name="ew_t")
                    # Schraudolph fast-exp disabled: DVE is the busier engine
                    # in steady state, and dropping it buys back error margin
                    if False:
                        # Schraudolph fast-exp on DVE: bf16-as-int16 bit
                        # trick, exp(SCALE*x) ~= bitcast(i16(A*x + B)).
                        # Offloads 1/4 of the exp wall off the Scalar engine.
                        nc.vector.tensor_scalar(
                            ew_t[:].bitcast(mybir.dt.int16),
                            _flat2(ps_sc),
                            EXP_A,
                            EXP_B,
                            op0=ALU.mult,
                            op1=ALU.add,
                        )
                    else:
                        nc.scalar.activation(
                            ew_t[:], _flat2(ps_sc), ACTF.Exp, scale=TOK_EXP_SCALE
                        )
                    tiles.append(ew_t)  # order: (g, j)
            ew_store[pidx] = tiles

        def emit_s3_half(pidx, hs):
            hp, npair = pidx_map(pidx)
            tiles = ew_store[pidx] if hs == 0 else ew_store.pop(pidx)
            h = 2 * hp + hs
            s3t = s3pool.tile([128, 2, 512], FP, tag="s3", name="s3t")
            s3v = s3t.rearrange("p g (qo x) -> p g qo x", x=128)
            for g in range(2):
                n = npair * 2 + g
                for qo in range(4):
                    for j in range(2):
                        tcg = 2 * n + j
                        nc.tensor.matmul(
                            s3v[:, g, qo, 0 : DV + 1],
                            tiles[2 * g + j][
                                :, hs * 512 + qo * 128 : hs * 512 + (qo + 1) * 128
                            ],
                            vw_sb[:, tcg, h, :],
                            start=(j == 0),
                            stop=(j == 1),
                        )
            # factor = attn_s / denom for the 2x4 tiles of this head
            # (DVE has no tensor_tensor divide op on this target)
            rec_t = smpool.tile([128, 2, 4], FP, tag="rec_t")
            nc.vector.reciprocal(rec_t[:], s3v[:, :, :, DV])
            fac_t = smpool.tile([128, 2, 4], FP, tag="fac_t")
            nc.vector.tensor_mul(
                fac_t[:],
                rec_t[:],
                attn_sb[:, :, h, npair * 2 : npair * 2 + 2].rearrange(
                    "p qo g -> p g qo"
                ),
            )
            fap = fac_t[:]
            fac_bc = bass.AP(
                fap.tensor, fap.offset, [list(x) for x in fap.ap] + [[0, DV]]
            )
            tmp_t = tmppool.tile([128, 2, 4, DV], BF, tag="tmp", name="tmp_t")
            nc.vector.tensor_tensor(tmp_t[:], s3v[:, :, :, 0:DV], fac_bc, ALU.mult)
            # ctx accumulation on the otherwise-idle gpsimd engine
            acc_sl = ctx_acc[:, :, h * 64 : (h + 1) * 64]
            if npair == 0:
                nc.gpsimd.tensor_tensor(acc_sl, tmp_t[:, 0], tmp_t[:, 1], ALU.add)
            else:
                nc.gpsimd.tensor_tensor(acc_sl, acc_sl, tmp_t[:, 0], ALU.add)
                nc.gpsimd.tensor_tensor(acc_sl, acc_sl, tmp_t[:, 1], ALU.add)

        def emit_head_tail(hp):
            # b_vw is folded into vw_sb, so the tail is only the ctx_w
            # transpose for the finished head pair.
            dc = hp
            ps = s3pool.tile([128, 2, 512], FP, tag="s3", name="ctps")
            for qo in range(4):
                nc.tensor.transpose(
                    ps[:, 0, qo * 128 : (qo + 1) * 128],
                    ctx_acc[:, qo, dc * 128 : (dc + 1) * 128],
                    ident[:],
                )
            nc.vector.tensor_copy(ctx_wT[:, dc, :], ps[:, 0, :])

        fc_ps = []

        def emit_fc_part1():
            # final fc, cc chunks 0-6 (fc1T + ctx_wT dc0-2): all inputs ready
            # by pidx 31; only dc3 (cc7) must wait for the last head.
            for mp in range(2):
                ps = scpool.tile([128, 2, 512], FP, tag="sc", name="fcp1")
                fc_ps.append(ps)
                for half in range(2):
                    mo = mp * 2 + half
                    for cc in range(4):
                        nc.tensor.matmul(
                            ps[:, half, :],
                            Wfc_sb[:, cc, mo * 128 : (mo + 1) * 128],
                            fc1T_sb[:, cc, :],
                            start=(cc == 0),
                            stop=False,
                        )
                    for dc in range(3):
                        nc.tensor.matmul(
                            ps[:, half, :],
                            Wfc_sb[:, 4 + dc, mo * 128 : (mo + 1) * 128],
                            ctx_wT[:, dc, :],
                            start=False,
                            stop=False,
                        )

        HP = H // 2

        # sg1-3 projections spread across the loop (the two-sweep pidx_map
        # defers the first sg2/sg3 use to pidx 16), so their PE matmuls and
        # DVE evictions don't pile up inside head-pair 0
        proj_sched = {
            4: lambda: kw_compute(1),
            6: lambda: vw_compute(1),
            10: lambda: kw_compute(2),
            12: lambda: vw_compute(2),
            18: lambda: kw_compute(3),
            20: lambda: vw_compute(3),
        }

        for pidx in range(HP * NP + 4):
            # priority order inside an iteration: s3 first (frees ew-ring
            # slots the exps are waiting on), then scores, then the
            # deadline-free projections
            if pidx >= 2 and pidx - 2 < HP * NP:
                emit_s3_half(pidx - 2, 0)
                emit_s3_half(pidx - 2, 1)
            if pidx in proj_sched:
                proj_sched[pidx]()
            if pidx < HP * NP:
                emit_scores(pidx)
            for st in startup_steps.get(pidx, ()):
                st()
            if 7 <= pidx <= 21 and pidx % 2 == 1:
                sentence_tail((pidx - 7) // 2)
            if pidx >= 28 and (pidx - 28) % 2 == 0:
                emit_head_tail((pidx - 28) // 2)
        # fc_part1 after the loop: its scpool tiles persist into phase 6, and
        # its 28 matmuls must rank below the final s3/tail chain so they fill
        # PE idle time instead of delaying the tail
        emit_fc_part1()

        aspool_cm.__exit__(None, None, None)
        t1pool_cm.__exit__(None, None, None)
        tmppool_cm.__exit__(None, None, None)
        apool_cm.__exit__(None, None, None)
        stpool_cm.__exit__(None, None, None)

        # ---------------- phase 6: sentence-ctx tail, then final fc
        lpool_cm = tc.tile_pool(name="late", bufs=1)
        lpool = lpool_cm.__enter__()
        outT_sb = lpool.tile([128, 4, LQ], BF, tag="outT_sb")
        # all 4 dc3 matmuls back-to-back (distinct PSUM banks), then the
        # evictions + output DMAs, so the tail chain isn't MM/evict ping-pong
        for mo in range(4):
            nc.tensor.matmul(
                fc_ps[mo // 2][:, mo % 2, :],
                Wfc_sb[:, 7, mo * 128 : (mo + 1) * 128],
                ctx_wT[:, 3, :],
                start=False,
                stop=True,
            )
        for mo in range(4):
            # split the four evictions between DVE and the (now idle) Scalar
            # engine so the tail chain isn't serialized on one engine
            if mo % 2 == 0:
                nc.vector.tensor_scalar_add(
                    outT_sb[:, mo, :], fc_ps[mo // 2][:, mo % 2, :],
                    btiles["bfcT"][:, mo : mo + 1],
                )
            else:
                nc.scalar.activation(
                    outT_sb[:, mo, :], fc_ps[mo // 2][:, mo % 2, :],
                    ACTF.Identity, bias=btiles["bfcT"][:, mo : mo + 1],
                )
            (nc.sync if mo % 2 == 0 else nc.scalar).dma_start(
                out=outT_d[mo * 128 : (mo + 1) * 128, :], in_=outT_sb[:, mo, :]
            )

        lpool_cm.__exit__(None, None, None)
        smpool_cm.__exit__(None, None, None)
        ewpool_cm.__exit__(None, None, None)
        s3pool_cm.__exit__(None, None, None)
        scpool_cm.__exit__(None, None, None)
        ppool_cm.__exit__(None, None, None)

    if not for_sim:
        ns = _split_multi_waits(nc)
        print(f"[kernel] split {ns} extra sem waits onto NOPs", file=sys.stderr)
    return nc


_NC_CACHE = None


def _get_nc():
    global _NC_CACHE
    if _NC_CACHE is None:
        _NC_CACHE = build_program()
    return _NC_CACHE


def make_in_maps(inputs):
    f = lambda x: np.ascontiguousarray(np.asarray(x, dtype=np.float32))
    q, k_w, v_w, k_s, v_s = (f(inputs[n]) for n in ["q", "k_w", "v_w", "k_s", "v_s"])
    W = {n: f(inputs[n]) for n in inputs if n.startswith(("W_", "b_"))}

    def bT(v, scale=1.0):
        return np.ascontiguousarray((v * scale).reshape(4, 128).T)

    def tl(a, kc=4):
        return np.ascontiguousarray(a.reshape(kc, 128, a.shape[-1]).transpose(1, 0, 2))

    bf = ml_dtypes.bfloat16
    f8 = ml_dtypes.float8_e4m3
    # fp8 weights scaled x16 so std~0.02 entries stay in e4m3's normal range
    WSC = 16.0
    shared = {
        "Wqs": tl((W["W_qs"] * WSC).astype(f8)),
        "Wks": tl((W["W_ks"] * WSC).astype(f8)),
        "Wvs": tl(W["W_vs"].astype(bf)),
        "Wqw": tl((W["W_qw"] * WSC).astype(f8)),
        "Wkw": tl((W["W_kw"] * WSC).astype(f8)),
        "Wvw": tl((W["W_vw"] * WSC).astype(f8)),
        "Wfc1": tl(W["W_fc1"].astype(bf)), "Wfc": tl(W["W_fc"].astype(bf), kc=8),
        "bqsT": bT(W["b_qs"], WSC), "bksT": bT(W["b_ks"]),
        "bqwT": bT(W["b_qw"], WSC),
        "bkwT": bT(W["b_kw"]), "bvsT": bT(W["b_vs"]),
        "bfc1T": bT(W["b_fc1"], 0.5), "bfcT": bT(W["b_fc"], 0.5),
        "bvw": W["b_vw"] * WSC,
    }
    in_maps = []
    for c in range(N_CORES):
        b, half = divmod(c, 2)
        blk = slice(half * NBH, half * NBH + NBH)
        ks_r = np.roll(k_s[b], -half * NBH, axis=0)
        vs_r = np.roll(v_s[b], -half * NBH, axis=0)
        m = dict(shared)
        m["qT"] = tl(q[b].T.astype(bf))
        kwT = k_w[b, blk].reshape(NTOK, D).T.astype(f8)
        vwT = v_w[b, blk].reshape(NTOK, D).T.astype(f8)

        def stg(a):
            a4 = a.reshape(4, 128, 4, 1024)            # [k, p, sg, t]
            return np.ascontiguousarray(a4.transpose(2, 1, 0, 3).reshape(4, 128, NTOK))

        m["kwT"] = stg(kwT)
        m["vwT"] = stg(vwT)
        m["ksT"] = tl(ks_r.T.astype(bf))
        m["vsT"] = tl(vs_r.T.astype(bf))
        in_maps.append(m)
    return in_maps


def run_cores(inputs, trace=False):
    nc = _get_nc()
    in_maps = make_in_maps(inputs)
    res = run_bass_kernel_spmd(nc, in_maps, list(range(N_CORES)), trace=trace)
    return res


def assemble(res):
    out = np.empty((B, LQ, D), dtype=np.float32)
    for b in range(B):
        out[b] = (
            res.results[2 * b]["outT"].astype(np.float32)
            + res.results[2 * b + 1]["outT"].astype(np.float32)
        ).T
    return out


def kernel(**inputs) -> np.ndarray:
    res = run_cores(inputs, trace=False)
    return assemble(res)


if __name__ == "__main__":
    import reference

    inp = {k: np.asarray(v) for k, v in reference.setup_inputs().items()}
    out = kernel(**inp)
    exp = np.asarray(reference.reference(**inp))
    err = np.abs(out - exp).max() / np.abs(exp).max()
    print("max rel err:", err)



# revision 86
# speedup vs baseline: 1.0190x; 1.0148x over previous
"""
Trainium2 Bass kernel for nn_MultiHeadHierarchicalAttention.

Sharding: 8 cores = (batch b in 0..3) x (block-half in 0..1).
Each core handles one batch and 16 of the 32 blocks for the token-level
attention; the (small) sentence-level attention + fc1 branch is computed
redundantly on both cores of a batch, scaled by 0.5, and the host sums the
two per-batch partial outputs (the final fc is linear, so partial ctx_w
contributions simply add).

Pipeline: (head-pair, block-pair) main loop in TWO SWEEPS (each head-pair
covers block-pairs 0-3 in sweep one, 4-7 in sweep two) so the sg2/sg3
kw/vw projections spread across the loop middle instead of piling into
head-pair 0; skew-2 scores->exp->s3; Scalar engine reserved for exp (the
~1.07us/tile exp stream is the kernel's wall); ctx accumulation on GpSimd;
deferred sentence branch overlapping the startup DMA trickle; final fc
emitted after the loop so it ranks below the tail chain.

Key tricks:
  - score matmuls are K=64 row-tiled PAIRS (tile_position 0/64) running
    concurrently in the PE array (~2x vs serial K=128 with padded qw).
  - fp8 weights (W_kw/W_vw/W_qw/W_qs/W_ks) are x16-scaled on the host so
    std-0.02 entries stay in e4m3's normal range; the 1/256 is folded into
    the exp scale and the vw ones-column is 16.0 so the softmax factor
    self-corrects. Mixed fp8-lhsT x bf16-rhs matmuls are used directly.
  - startup DMAs ride the sync+scalar hardware rings (FIFO = priority,
    ~215 GB/s pooled) with few triggers (~0.7us each); small bias loads go
    before bulky weights on gpsimd's ~85 GB/s software queue.

Device layouts (per core, partition dim first):
  qT   [D, LQ]      kwT/vwT [D, 4096]   ksT/vsT [D, 32] (block-rolled)
  projections keep features on partitions (kw) or tokens on partitions (vw)
  token scores are computed as [t, q] tiles (K=dk=64), exp'd on ACT into
  bf16, and consumed per block by S3 matmuls producing [q, dv+1] partials
  (the +1 "ones" column of vw gives the softmax denominator for free).
  The sentence-attention factor attn_s/denom scales the per-block ctx on
  DVE; b_vw is folded into vw_sb so no separate bias pass is needed.
  ctx_w is PE-transposed at the end and fused into the final fc, which is
  emitted as outT [D, LQ] (host transposes and sums core pairs).
"""

import sys

sys.path.insert(0, "/opt/trn_rl_repo")

import numpy as np
import ml_dtypes
import concourse.bass as bass
import concourse.tile as tile
from concourse import mybir
from concourse.bass_utils import run_bass_kernel_spmd
from concourse.vector_clock import ScopedClock
from concourse.masks import make_identity

# ---------------------------------------------------------------- constants
B, LQ, NB, NT = 4, 512, 32, 256
D, H, DK, DV = 512, 8, 64, 64
NBH = NB // 2  # blocks per core
NTOK = NBH * NT  # tokens per core = 4096
NTC = NTOK // 128  # 32 token chunks of 128
SCALE = 0.125
# fp8 weights are scaled x16 on the host so W std ~0.02 lands in e4m3's
# normal range (raw 0.02-std weights round through subnormals at huge
# relative error). The projections then produce 16x-scaled kw/qw/qs/ks;
# evictions and exp scales fold the 1/16 back in.
WSC = 16.0
TOK_EXP_SCALE = SCALE / (WSC * WSC)  # qw and kw both carry x16
# Schraudolph fast-exp (bf16-as-int16): exp(s*x) ~= bitcast(i16(A*x+B))
EXP_A = 128 * 1.4426950408889634 * TOK_EXP_SCALE
EXP_B = 16250.5
FP = mybir.dt.float32
FR = mybir.dt.float32r
BF = mybir.dt.bfloat16
F8 = mybir.dt.float8e4
N_CORES = 8

AX = mybir.AxisListType.X
ALU = mybir.AluOpType
ACTF = mybir.ActivationFunctionType


# --------------------------------------------------------- drain workaround
def _patched_drain_and_barrier(self, tick_clock, wait_clock):
    # walrus in this container rejects >1 sem wait on a single TPB_CTRL
    # instruction ("Too many sync wait commands"); split the kernel-tail
    # drain waits across one-wait NOPs.
    nop_inst = self.nc.sync.nop(nofuse=True)
    wait_clock.add_sem_waits(nop_inst.ins, ScopedClock({None: tick_clock.global_clock}))
    waits = list(nop_inst.ins.sync_info.on_wait or [])
    if len(waits) > 1:
        nop_inst.ins.sync_info.on_wait = waits[:1]
        rest = waits[1:]
        while rest:
            extra = self.nc.sync.nop(nofuse=True)
            if extra.ins.sync_info is None:
                extra.ins.sync_info = mybir.SyncInfo(on_wait=[], on_update=[])
            extra.ins.sync_info.on_wait = rest[:1]
            rest = rest[1:]
    self.nc.sync.drain()
    self.nc.all_engine_barrier()
    assert self.sems is not None
    popped = self.nc._tile_sem_poison_stack.pop()
    assert popped is self._sem_poison
    self.nc.clear_and_free_semaphores(list(self.sems.allocated().values()))
    self.nc.all_engine_barrier()


_ORIG_DRAIN_AND_BARRIER = tile.TileContext._drain_and_barrier
tile.TileContext._drain_and_barrier = _patched_drain_and_barrier


def _r(ap):
    """View an f32 AP as float32r so matmuls run at 1 cycle/row."""
    return ap.bitcast(mybir.dt.float32r)


_NO_SPLIT_OPCODES = {
    "CollectiveCompute",
    "EventSemaphore",
}
_split_counter = [0]


def _split_multi_waits(nc):
    """This container's walrus accepts at most ONE sem wait per TPB
    instruction; hoist extra waits onto same-engine NOPs placed before."""
    n_split = 0
    for fn in nc.m.functions:
        for bb in fn.blocks:
            changed = False
            out = []
            for inst in bb.instructions:
                si = inst.sync_info
                if (
                    si is not None
                    and si.on_wait
                    and len(list(si.on_wait)) > 1
                    and inst.opcode not in _NO_SPLIT_OPCODES
                ):
                    waits = list(si.on_wait)
                    for w in waits[:-1]:
                        _split_counter[0] += 1
                        nop = mybir.InstNoOp(name=f"I-wsplit-{_split_counter[0]}")
                        nop.engine = inst.engine
                        nop.sync_info = mybir.SyncInfo(on_wait=[w], on_update=[])
                        out.append(nop)
                        n_split += 1
                    si.on_wait = waits[-1:]
                    changed = True
                out.append(inst)
            if changed:
                bb.instructions = out
    return n_split


def _flat2(ap):
    """[p, a, b] -> [p, a*b]"""
    return ap.rearrange("p a b -> p (a b)")


# ------------------------------------------------------------ program build
def build_program(for_sim=False):
    # the walrus-only wait-splitting workarounds confuse CoreSim's race
    # detector; skip them when building for simulation.
    tile.TileContext._drain_and_barrier = (
        _ORIG_DRAIN_AND_BARRIER if for_sim else _patched_drain_and_barrier
    )
    nc = bass.Bass("TRN2", target_bir_lowering=False, debug=False, num_devices=N_CORES)

    dt_in = {}
    for name, shape in [
        ("kwT", [4, 128, NTOK]),
        ("vwT", [4, 128, NTOK]),
        ("Wkw", [128, 4, H * DK]),
        ("Wvw", [128, 4, H * DV]),
        ("Wqs", [128, 4, H * DK]),
        ("Wks", [128, 4, H * DK]),
        ("Wqw", [128, 4, H * DK]),
    ]:
        dt_in[name] = nc.dram_tensor(name, shape, F8, kind="ExternalInput").ap()
    for name, shape in [
        ("qT", [128, 4, LQ]),
        ("ksT", [128, 4, NB]),
        ("vsT", [128, 4, NB]),
        ("Wvs", [128, 4, H * DV]),
        ("Wfc", [128, 8, D]),
        ("Wfc1", [128, 4, D]),
    ]:
        dt_in[name] = nc.dram_tensor(name, shape, BF, kind="ExternalInput").ap()
    for name, shape in [
        ("bqsT", [128, 4]),
        ("bksT", [128, 4]),
        ("bqwT", [128, 4]),
        ("bkwT", [128, 4]),
        ("bvsT", [128, 4]),
        ("bfc1T", [128, 4]),
        ("bfcT", [128, 4]),
        ("bvw", [H * DV]),
    ]:
        dt_in[name] = nc.dram_tensor(name, shape, FP, kind="ExternalInput").ap()
    outT_d = nc.dram_tensor("outT", [D, LQ], BF, kind="ExternalOutput").ap()

    with tile.TileContext(nc) as tc:
        # ------------------------------------------------ persistent pools
        ppool_cm = tc.tile_pool(name="persist", bufs=1)
        ppool = ppool_cm.__enter__()
        scpool_cm = tc.tile_pool(name="scps", bufs=2, space="PSUM")
        scpool = scpool_cm.__enter__()
        s3pool_cm = tc.tile_pool(name="s3ps", bufs=2, space="PSUM")
        s3pool = s3pool_cm.__enter__()
        ewpool_cm = tc.tile_pool(name="ewp", bufs=18)
        ewpool = ewpool_cm.__enter__()
        smpool_cm = tc.tile_pool(name="small", bufs=8)
        smpool = smpool_cm.__enter__()

        ident = ppool.tile([128, 128], FP, tag="ident")

        # persistent sbuf tensors
        qw_pad = ppool.tile([128, H, LQ], BF, tag="qw_pad")
        ks_sb = ppool.tile([128, 4, NB], BF, tag="ks_sb")
        attn_sb = ppool.tile([128, 4, H, NB], FP, tag="attn_sb")
        fc1T_sb = ppool.tile([128, 4, LQ], BF, tag="fc1T")
        kw_sb = ppool.tile([128, 4, NTOK], BF, tag="kw_sb")
        vw_sb = ppool.tile([128, NTC, H, DV + 1], BF, tag="vw_sb")
        ctx_acc = ppool.tile([128, 4, H * DV], FP, tag="ctx_acc")
        vs_sb = ppool.tile([NB, H * DV], BF, tag="vs_sb")
        ctx_sT = ppool.tile([128, 4, LQ], BF, tag="ctx_sT")

        # staging pool + kw/vw weight tiles created early so the big DMAs
        # stream during the small branch.
        stpool_cm = tc.tile_pool(name="stage", bufs=3)
        stpool = stpool_cm.__enter__()
        Wkw_sb = ppool.tile([128, 4, H * DK], F8, tag="Wkw")
        Wvw_sb = ppool.tile([128, 4, H * DV], F8, tag="Wvw")
        ctx_wT = ppool.tile([128, 4, LQ], BF, tag="ctx_wT")
        stg_store = {}

        def dma_halves(tile4, in_ap, eng0, eng1):
            # split a [128, 4, n] load across two DMA queues: one queue tops
            # out around ~70 GB/s, which serializes the startup badly.
            eng0.dma_start(out=tile4[:, 0:2, :], in_=in_ap[:, 0:2, :])
            eng1.dma_start(out=tile4[:, 2:4, :], in_=in_ap[:, 2:4, :])

        def stage_dma(sg, engs=None):
            kstg = stpool.tile([128, 4, 1024], F8, tag="kstg", name="kstg")
            vstg = stpool.tile([128, 4, 1024], F8, tag="vstg", name="vstg")
            if engs is None:
                nc.sync.dma_start(
                    out=kstg.rearrange("p a b -> p (a b)"), in_=dt_in["kwT"][sg]
                )
                nc.sync.dma_start(
                    out=vstg.rearrange("p a b -> p (a b)"), in_=dt_in["vwT"][sg]
                )
            else:
                ka = dt_in["kwT"][sg].rearrange("p (a b) -> p a b", b=1024)
                va = dt_in["vwT"][sg].rearrange("p (a b) -> p a b", b=1024)
                dma_halves(kstg, ka, engs[0], engs[1])
                dma_halves(vstg, va, engs[2], engs[3])
            stg_store[sg] = (kstg, vstg)

        DR = mybir.MatmulPerfMode.DoubleRow

        def kw_compute(sg):
            stg = stg_store[sg][0]
            for mo in range(4):
                ps = s3pool.tile([128, 2, 512], FP, tag="s3", name="kwps")
                for j in range(2):
                    for s in range(2):
                        nc.tensor.matmul(
                            ps[:, j, :],
                            Wkw_sb[:, 2 * s : 2 * s + 2, mo * 128 : (mo + 1) * 128],
                            stg[:, 2 * s : 2 * s + 2, j * 512 : (j + 1) * 512],
                            start=(s == 0),
                            stop=(s == 1),
                            perf_mode=DR,
                        )
                # NOTE: b_kw adds a per-(h,q) constant to every score in every
                # block; it cancels in the within-block softmax and in the
                # attn_s/denom ratio, so it is dropped entirely.
                if sg == 0:
                    # startup: ACT is idle until the first exp (~25us), so
                    # the sg0 evictions run there, off the DVE critical
                    # path that gates the first score matmuls
                    nc.scalar.activation(
                        kw_sb[:, mo, sg * 1024 : (sg + 1) * 1024],
                        _flat2(ps),
                        ACTF.Copy,
                    )
                else:
                    nc.vector.tensor_copy(
                        kw_sb[:, mo, sg * 1024 : (sg + 1) * 1024], _flat2(ps)
                    )

        def vw_compute(sg):
            stg = stg_store[sg][1]
            for tp in range(4):
                ps = s3pool.tile([128, 2, 512], FP, tag="s3", name="vwps")
                for j in range(2):
                    tcl = tp * 2 + j
                    for s in range(2):
                        nc.tensor.matmul(
                            ps[:, j, :],
                            stg[:, 2 * s : 2 * s + 2, tcl * 128 : (tcl + 1) * 128],
                            Wvw_sb[:, 2 * s : 2 * s + 2, :],
                            start=(s == 0),
                            stop=(s == 1),
                            perf_mode=DR,
                        )
                tc0 = sg * 8 + tp * 2
                # fold b_vw into vw: the per-block softmax-weighted sum of
                # (vw + b) equals ctx + denom*b, and the attn_s/denom factor
                # turns that into exactly the b_vw bias term summed over
                # blocks — no separate head-tail bias pass needed.
                bap = bvw_bc[:].rearrange("p (h x) -> p h x", x=DV)
                bvw_view = bass.AP(
                    bap.tensor,
                    bap.offset,
                    [list(bap.ap[0])] + [[0, 2]] + [list(d) for d in bap.ap[1:]],
                )
                nc.vector.tensor_tensor(
                    vw_sb[:, tc0 : tc0 + 2, :, 0:DV],
                    ps.rearrange("p a (h x) -> p a h x", x=DV),
                    bvw_view,
                    ALU.add,
                )

        # ------------------------------------------- phase 1: token-path start
        # Critical path: qT+Wqw -> qw proj -> kw sg0 -> first scores -> exp.
        # The sentence branch is deferred into the first two main-loop
        # iterations so ACT starts exp'ing as early as possible.
        apool_cm = tc.tile_pool(name="phaseA", bufs=1)
        apool = apool_cm.__enter__()

        qT_sb = apool.tile([128, 4, LQ], BF, tag="qT_sb")
        ksT_sb = apool.tile([128, 4, NB], BF, tag="ksT_sb")
        vsT_sb = apool.tile([128, 4, NB], BF, tag="vsT_sb")
        qs_sb = apool.tile([128, 4, LQ], BF, tag="qs_sb")

        def load_W(wn, wdt=BF, eng=None, engs=None):
            t = apool.tile([128, 4, D], wdt, tag=f"W_{wn}", name=wn)
            if engs is not None:
                dma_halves(t, dt_in[wn], engs[0], engs[1])
            else:
                (eng or nc.gpsimd).dma_start(out=t[:], in_=dt_in[wn][:])
            return t

        # Startup DMAs. Each dma_start trigger costs ~0.7us on the issuing
        # engine, and transfers stripe across the 16 DMA engines regardless,
        # so: few triggers, hardware ring (sync) drains ~215 GB/s FIFO in
        # emission order = priority, gpsimd's software queue (~85 GB/s) runs
        # the fp8 weights + small loads concurrently. Nothing on scalar --
        # the exp wall lives there.
        # the scalar engine is idle until the first exp (~20us), so its DMA
        # ring carries half of the startup-critical loads for free
        dma_halves(qT_sb, dt_in["qT"], nc.sync, nc.scalar)
        Wqw_t = load_W("Wqw", F8)
        # tiny loads (biases feed the qw/qs evictions!) must not queue behind
        # the bulky fp8 weights on gpsimd's slow software queue
        btiles = {}
        for bn in ["bqsT", "bqwT", "bvsT", "bfc1T", "bfcT"]:
            t = ppool.tile([128, 4], FP, tag=bn)
            nc.gpsimd.dma_start(out=t[:], in_=dt_in[bn][:])
            btiles[bn] = t
        bvw_bc = ppool.tile([128, H * DV], FP, tag="bvw_bc")
        src = dt_in["bvw"]
        bcast_ap = bass.AP(src.tensor, src.offset, [[0, 128]] + [list(x) for x in src.ap])
        nc.gpsimd.dma_start(out=bvw_bc[:], in_=bcast_ap)
        nc.gpsimd.dma_start(out=ksT_sb[:], in_=dt_in["ksT"][:])
        nc.gpsimd.dma_start(out=vsT_sb[:], in_=dt_in["vsT"][:])
        nc.sync.dma_start(out=Wkw_sb[:], in_=dt_in["Wkw"][:])
        stage_dma(0, engs=(nc.sync, nc.scalar, nc.sync, nc.scalar))
        nc.sync.dma_start(out=Wvw_sb[:], in_=dt_in["Wvw"][:])
        Wqs_t = load_W("Wqs", F8, eng=nc.sync)
        Wks_t = load_W("Wks", F8, eng=nc.sync)
        make_identity(nc, ident)
        nc.vector.memset(vw_sb[:, :, :, DV : DV + 1], WSC)
        # PE warm-up: ~4us of dummy matmuls while the startup DMAs stream,
        # so the HAM clock-gate reaches 2.4 GHz before the real projections
        # (otherwise qw/kw0 run at the cold 1.2 GHz rate)
        wu_sb = apool.tile([128, 128], BF, tag="warmup")
        nc.vector.memset(wu_sb[:], 0.0)
        wu_ps = scpool.tile([128, 512], FP, tag="sc", name="wups")
        for i in range(32):
            nc.tensor.matmul(
                wu_ps[:, 0:128], wu_sb[:], wu_sb[:], start=(i == 0), stop=(i == 31)
            )

        # qw projection into per-head layout: head h=2mo on partitions
        # 0:64 of plane 2mo, h=2mo+1 on 64:128 of plane 2mo+1; the score
        # matmuls are row-tiled K=64 so the unwritten halves are never
        # read.
        for mo in range(4):
            ps = scpool.tile([128, 512], FP, tag="sc", name="qwps")
            for k in range(4):
                nc.tensor.matmul(
                    ps[:],
                    Wqw_t[:, k, mo * 128 : (mo + 1) * 128],
                    qT_sb[:, k, :],
                    start=(k == 0),
                    stop=(k == 3),
                )
            nc.vector.tensor_scalar_add(
                qw_pad[0:64, 2 * mo, :], ps[0:64, :], btiles["bqwT"][0:64, mo : mo + 1]
            )
            nc.vector.tensor_scalar_add(
                qw_pad[64:128, 2 * mo + 1, :],
                ps[64:128, :],
                btiles["bqwT"][64:128, mo : mo + 1],
            )
        kw_compute(0)
        vw_compute(0)
        stage_dma(1, engs=(nc.sync, nc.scalar, nc.sync, nc.scalar))
        Wvs_t = load_W("Wvs", eng=nc.sync)
        stage_dma(2)
        stage_dma(3)

        # --------------------- deferred sentence branch (runs at pidx 0-1)
        ews_all = apool.tile([128, 4, H, NB], FP, tag="ews_all")
        sden = apool.tile([128, 4, H], FP, tag="sden")
        srec = apool.tile([128, 4, H], FP, tag="srec")

        def sent_qs():
            for mo in range(4):
                ps = scpool.tile([128, 512], FP, tag="sc", name="qsps")
                for k in range(4):
                    nc.tensor.matmul(
                        ps[:],
                        Wqs_t[:, k, mo * 128 : (mo + 1) * 128],
                        qT_sb[:, k, :],
                        start=(k == 0),
                        stop=(k == 3),
                    )
                nc.vector.tensor_scalar_add(
                    qs_sb[:, mo, :], ps[:], btiles["bqsT"][:, mo : mo + 1]
                )

        def sent_ks():
            # ks projection: out [hdk(mo), nb]; b_ks cancels in the softmax.
            for mo in range(4):
                ps = scpool.tile([128, 512], FP, tag="sc", name="ksps")
                for k in range(4):
                    nc.tensor.matmul(
                        ps[:, 0:NB],
                        Wks_t[:, k, mo * 128 : (mo + 1) * 128],
                        ksT_sb[:, k, :],
                        start=(k == 0),
                        stop=(k == 3),
                    )
                nc.scalar.activation(ks_sb[:, mo, :], ps[:, 0:NB], ACTF.Copy)

        def sent_vs():
            # vs projection: out [nb, hdv]  (lhsT = vsT chunk, rhs = Wvs)
            ps = scpool.tile([128, 512], FP, tag="sc", name="vsps")
            for k in range(4):
                nc.tensor.matmul(
                    ps[0:NB, :],
                    vsT_sb[:, k, :],
                    Wvs_t[:, k, :],
                    start=(k == 0),
                    stop=(k == 3),
                )
            nc.scalar.activation(vs_sb[:, :], ps[0:NB, :], ACTF.Copy)

        def sent_attn(h0, h1):
            for h in range(h0, h1):
                hp, po = h // 2, (h % 2) * 64
                ps = scpool.tile([128, 512], FP, tag="sc", name="sattps")
                for qo in range(4):
                    nc.tensor.matmul(
                        ps[:, qo * NB : (qo + 1) * NB],
                        qs_sb[po : po + 64, hp, qo * 128 : (qo + 1) * 128],
                        ks_sb[po : po + 64, hp, :],
                        start=True,
                        stop=True,
                    )
                nc.scalar.activation(
                    ews_all[:, :, h, :],
                    ps[:, 0 : 4 * NB].rearrange("p (a x) -> p a x", x=NB),
                    ACTF.Exp,
                    scale=TOK_EXP_SCALE,
                )

        def sent_softmax():
            # batched sentence softmax: one reduce / recip / bcast-mul
            nc.vector.tensor_reduce(sden[:], ews_all[:], AX, ALU.add)
            nc.vector.reciprocal(srec[:], sden[:])
            rap = srec[:]
            rec_bc = bass.AP(
                rap.tensor, rap.offset, [list(x) for x in rap.ap] + [[0, NB]]
            )
            # on DVE, not gpsimd: this mult gates the first s3 factors and
            # gpsimd is ~2x slower per op on the startup critical path
            nc.vector.tensor_tensor(attn_sb[:], ews_all[:], rec_bc, ALU.mult)

        # per-pidx deferred startup work, sequenced to the DMA landing times;
        # the whole sentence-softmax chain goes at pidx 0 so attn_sb (which
        # gates every s3 factor) completes as early as possible
        startup_steps = {
            0: [sent_qs, sent_ks, lambda: sent_attn(0, 8), sent_softmax],
            2: [sent_vs],
        }

        # ---------------- phase 3+4+5: projections interleaved with attention
        tmppool_cm = tc.tile_pool(name="tmpp", bufs=4)
        tmppool = tmppool_cm.__enter__()

        # token scores + ctx_w accumulation, software-pipelined with skew-2
        # between scores/exp and the s3 consumption.
        NP = NBH // 2  # pairs per head

        def pidx_map(pidx):
            # four sweeps of 8 pidx: each head-pair covers block-pairs
            # {2s, 2s+1} in sweep s, so sweep one touches only staging group
            # sg0 and sg s is first read at pidx 8s -- the score pipeline
            # never waits on the kwT/vwT DMA trickle, and the projection
            # evictions spread evenly across the loop
            sweep, r = divmod(pidx, 8)
            return r // 2, sweep * 2 + (r % 2)

        ew_store = {}
        tmp_store = {}
        t1pool_cm = tc.tile_pool(name="tail1", bufs=1)
        t1pool = t1pool_cm.__enter__()
        aspool_cm = tc.tile_pool(name="asTring", bufs=2)
        aspool = aspool_cm.__enter__()
        Wfc1_sb = t1pool.tile([128, 4, D], BF, tag="Wfc1l")
        nc.gpsimd.dma_start(out=Wfc1_sb[:], in_=dt_in["Wfc1"][:])
        Wfc_sb = t1pool.tile([128, 8, D], BF, tag="Wfc")
        nc.gpsimd.dma_start(out=Wfc_sb[:], in_=dt_in["Wfc"][:])

        def sentence_tail(h):
            hp, po = h // 2, (h % 2) * 64
            asT_h = aspool.tile([NB, 4, 128], BF, tag="asT", name="asT_h")
            psT = s3pool.tile([128, 2, 512], FP, tag="s3", name="sattT")
            for qo in range(4):
                nc.tensor.transpose(
                    psT[0:NB, 0, qo * 128 : (qo + 1) * 128],
                    attn_sb[:, qo, h, :],
                    ident[:],
                )
            nc.vector.tensor_copy(asT_h[:, :, :], psT[0:NB, 0, :])
            # ctx_sT [dv(h), q] = vs.T @ attn_sT (+ b_vs per-partition)
            ps_t = s3pool.tile([128, 2, 512], FP, tag="s3", name="ctxsps")
            ps = ps_t[:, 0, :]
            nc.tensor.matmul(
                ps[0:64, :],
                vs_sb[:, h * 64 : (h + 1) * 64],
                asT_h[:, :, :],
                start=True,
                stop=True,
            )
            nc.vector.tensor_scalar_add(
                ctx_sT[po : po + 64, hp, :],
                ps[0:64, :],
                btiles["bvsT"][po : po + 64, hp : hp + 1],
            )
            if h == H - 1:
                # fc1: out [dm(mo), q] = Wfc1.T @ ctx_sT, x0.5 (+0.5*b_fc1)
                for mo in range(4):
                    ps_t = s3pool.tile([128, 2, 512], FP, tag="s3", name="fc1ps")
                    ps = ps_t[:, 0, :]
                    for k in range(4):
                        nc.tensor.matmul(
                            ps[:],
                            Wfc1_sb[:, k, mo * 128 : (mo + 1) * 128],
                            ctx_sT[:, k, :],
                            start=(k == 0),
                            stop=(k == 3),
                        )
                    nc.vector.tensor_scalar(
                        fc1T_sb[:, mo, :],
                        ps[:],
                        0.5,
                        btiles["bfc1T"][:, mo : mo + 1],
                        op0=ALU.mult,
                        op1=ALU.add,
                    )

        def emit_scores(pidx):
            hp, npair = pidx_map(pidx)
            tiles = []
            for g in range(2):
                n = npair * 2 + g
                for j in range(2):
                    tcg = 2 * n + j
                    # row-tiled pair: the two heads' K=64 matmuls occupy
                    # disjoint row-groups (partitions 0:64 / 64:128) and run
                    # concurrently in the PE array (~2x vs serial K=128).
                    ps_sc = scpool.tile([128, 2, 512], FP, tag="sc", name="scps")
                    nc.tensor.matmul(
                        ps_sc[:, 0, :],
                        kw_sb[0:64, hp, tcg * 128 : (tcg + 1) * 128],
                        qw_pad[0:64, 2 * hp, :],
                        start=True,
                        stop=True,
                    )
                    nc.tensor.matmul(
                        ps_sc[:, 1, :],
                        kw_sb[64:128, hp, tcg * 128 : (tcg + 1) * 128],
                        qw_pad[64:128, 2 * hp + 1, :],
                        start=True,
                        stop=True,
                    )
                    ew_t = ewpool.tile([128, 1024], BF, tag="ew", name="ew_t")
                    # Schraudolph fast-exp on 1/4 of pidx: ACT (71% busy)
                    # now paces ahead of DVE (65%), so a light offload of the
                    # exp wall onto DVE rebalances the steady state
                    if g == 1 and j == 1 and pidx % 4 == 1:
                        # Schraudolph fast-exp on DVE: bf16-as-int16 bit
                        # trick, exp(SCALE*x) ~= bitcast(i16(A*x + B)).
                        # Offloads 1/4 of the exp wall off the Scalar engine.
                        nc.vector.tensor_scalar(
                            ew_t[:].bitcast(mybir.dt.int16),
                            _flat2(ps_sc),
                            EXP_A,
                            EXP_B,
                            op0=ALU.mult,
                            op1=ALU.add,
                        )
                    else:
                        nc.scalar.activation(
                            ew_t[:], _flat2(ps_sc), ACTF.Exp, scale=TOK_EXP_SCALE
                        )
                    tiles.append(ew_t)  # order: (g, j)
            ew_store[pidx] = tiles

        def emit_s3_half(pidx, hs):
            hp, npair = pidx_map(pidx)
            tiles = ew_store[pidx] if hs == 0 else ew_store.pop(pidx)
            h = 2 * hp + hs
            s3t = s3pool.tile([128, 2, 512], FP, tag="s3", name="s3t")
            s3v = s3t.rearrange("p g (qo x) -> p g qo x", x=128)
            for g in range(2):
                n = npair * 2 + g
                for qo in range(4):
                    for j in range(2):
                        tcg = 2 * n + j
                        nc.tensor.matmul(
                            s3v[:, g, qo, 0 : DV + 1],
                            tiles[2 * g + j][
                                :, hs * 512 + qo * 128 : hs * 512 + (qo + 1) * 128
                            ],
                            vw_sb[:, tcg, h, :],
                            start=(j == 0),
                            stop=(j == 1),
                        )
            # factor = attn_s / denom for the 2x4 tiles of this head
            # (DVE has no tensor_tensor divide op on this target)
            rec_t = smpool.tile([128, 2, 4], FP, tag="rec_t")
            nc.vector.reciprocal(rec_t[:], s3v[:, :, :, DV])
            fac_t = smpool.tile([128, 2, 4], FP, tag="fac_t")
            nc.vector.tensor_mul(
                fac_t[:],
                rec_t[:],
                attn_sb[:, :, h, npair * 2 : npair * 2 + 2].rearrange(
                    "p qo g -> p g qo"
                ),
            )
            fap = fac_t[:]
            fac_bc = bass.AP(
                fap.tensor, fap.offset, [list(x) for x in fap.ap] + [[0, DV]]
            )
            tmp_t = tmppool.tile([128, 2, 4, DV], BF, tag="tmp", name="tmp_t")
            nc.vector.tensor_tensor(tmp_t[:], s3v[:, :, :, 0:DV], fac_bc, ALU.mult)
            # ctx accumulation on the otherwise-idle gpsimd engine
            acc_sl = ctx_acc[:, :, h * 64 : (h + 1) * 64]
            if npair == 0:
                nc.gpsimd.tensor_tensor(acc_sl, tmp_t[:, 0], tmp_t[:, 1], ALU.add)
            else:
                nc.gpsimd.tensor_tensor(acc_sl, acc_sl, tmp_t[:, 0], ALU.add)
                nc.gpsimd.tensor_tensor(acc_sl, acc_sl, tmp_t[:, 1], ALU.add)

        def emit_head_tail(hp):
            # b_vw is folded into vw_sb, so the tail is only the ctx_w
            # transpose for the finished head pair.
            dc = hp
            ps = s3pool.tile([128, 2, 512], FP, tag="s3", name="ctps")
            for qo in range(4):
                nc.tensor.transpose(
                    ps[:, 0, qo * 128 : (qo + 1) * 128],
                    ctx_acc[:, qo, dc * 128 : (dc + 1) * 128],
                    ident[:],
                )
            nc.vector.tensor_copy(ctx_wT[:, dc, :], ps[:, 0, :])

        fc_ps = []

        def emit_fc_part1():
            # final fc, cc chunks 0-6 (fc1T + ctx_wT dc0-2): all inputs ready
            # by pidx 31; only dc3 (cc7) must wait for the last head.
            for mp in range(2):
                ps = scpool.tile([128, 2, 512], FP, tag="sc", name="fcp1")
                fc_ps.append(ps)
                for half in range(2):
                    mo = mp * 2 + half
                    for cc in range(4):
                        nc.tensor.matmul(
                            ps[:, half, :],
                            Wfc_sb[:, cc, mo * 128 : (mo + 1) * 128],
                            fc1T_sb[:, cc, :],
                            start=(cc == 0),
                            stop=False,
                        )
                    for dc in range(3):
                        nc.tensor.matmul(
                            ps[:, half, :],
                            Wfc_sb[:, 4 + dc, mo * 128 : (mo + 1) * 128],
                            ctx_wT[:, dc, :],
                            start=False,
                            stop=False,
                        )

        HP = H // 2

        # sg1-3 projections spread across the loop (the two-sweep pidx_map
        # defers the first sg2/sg3 use to pidx 16), so their PE matmuls and
        # DVE evictions don't pile up inside head-pair 0
        proj_sched = {
            4: lambda: kw_compute(1),
            6: lambda: vw_compute(1),
            10: lambda: kw_compute(2),
            12: lambda: vw_compute(2),
            18: lambda: kw_compute(3),
            20: lambda: vw_compute(3),
        }

        for pidx in range(HP * NP + 4):
            # priority order inside an iteration: s3 first (frees ew-ring
            # slots the exps are waiting on), then scores, then the
            # deadline-free projections
            if pidx >= 2 and pidx - 2 < HP * NP:
                emit_s3_half(pidx - 2, 0)
                emit_s3_half(pidx - 2, 1)
            if pidx in proj_sched:
                proj_sched[pidx]()
            if pidx < HP * NP:
                emit_scores(pidx)
            for st in startup_steps.get(pidx, ()):
                st()
            if 7 <= pidx <= 21 and pidx % 2 == 1:
                sentence_tail((pidx - 7) // 2)
            if pidx >= 28 and (pidx - 28) % 2 == 0:
                emit_head_tail((pidx - 28) // 2)
        # fc_part1 after the loop: its scpool tiles persist into phase 6, and
        # its 28 matmuls must rank below the final s3/tail chain so they fill
        # PE idle time instead of delaying the tail
        emit_fc_part1()

        aspool_cm.__exit__(None, None, None)
        t1pool_cm.__exit__(None, None, None)
        tmppool_cm.__exit__(None, None, None)
        apool_cm.__exit__(None, None, None)
        stpool_cm.__exit__(None, None, None)

        # ---------------- phase 6: sentence-ctx tail, then final fc
        lpool_cm = tc.tile_pool(name="late", bufs=1)
        lpool = lpool_cm.__enter__()
        outT_sb = lpool.tile([128, 4, LQ], BF, tag="outT_sb")
        # all 4 dc3 matmuls back-to-back (distinct PSUM banks), then the
        # evictions + output DMAs, so the tail chain isn't MM/evict ping-pong
        for mo in range(4):
            nc.tensor.matmul(
                fc_ps[mo // 2][:, mo % 2, :],
                Wfc_sb[:, 7, mo * 128 : (mo + 1) * 128],
                ctx_wT[:, 3, :],
                start=False,
                stop=True,
            )
        for mo in range(4):
            # split the four evictions between DVE and the (now idle) Scalar
            # engine so the tail chain isn't serialized on one engine
            if mo % 2 == 0:
                nc.vector.tensor_scalar_add(
                    outT_sb[:, mo, :], fc_ps[mo // 2][:, mo % 2, :],
                    btiles["bfcT"][:, mo : mo + 1],
                )
            else:
                nc.scalar.activation(
                    outT_sb[:, mo, :], fc_ps[mo // 2][:, mo % 2, :],
                    ACTF.Identity, bias=btiles["bfcT"][:, mo : mo + 1],
                )
            (nc.sync if mo % 2 == 0 else nc.scalar).dma_start(
                out=outT_d[mo * 128 : (mo + 1) * 128, :], in_=outT_sb[:, mo, :]
            )

        lpool_cm.__exit__(None, None, None)
        smpool_cm.__exit__(None, None, None)
        ewpool_cm.__exit__(None, None, None)
        s3pool_cm.__exit__(None, None, None)
        scpool_cm.__exit__(None, None, None)
        ppool_cm.__exit__(None, None, None)

    if not for_sim:
        ns = _split_multi_waits(nc)
        print(f"[kernel] split {ns} extra sem waits onto NOPs", file=sys.stderr)
    return nc


_NC_CACHE = None


def _get_nc():
    global _NC_CACHE
    if _NC_CACHE is None:
        _NC_CACHE = build_program()
    return _NC_CACHE


def make_in_maps(inputs):
    f = lambda x: np.ascontiguousarray(np.asarray(x, dtype=np.float32))
    q, k_w, v_w, k_s, v_s = (f(inputs[n]) for n in ["q", "k_w", "v_w", "k_s", "v_s"])
    W = {n: f(inputs[n]) for n in inputs if n.startswith(("W_", "b_"))}

    def bT(v, scale=1.0):
        return np.ascontiguousarray((v * scale).reshape(4, 128).T)

    def tl(a, kc=4):
        return np.ascontiguousarray(a.reshape(kc, 128, a.shape[-1]).transpose(1, 0, 2))

    bf = ml_dtypes.bfloat16
    f8 = ml_dtypes.float8_e4m3
    # fp8 weights scaled x16 so std~0.02 entries stay in e4m3's normal range
    WSC = 16.0
    shared = {
        "Wqs": tl((W["W_qs"] * WSC).astype(f8)),
        "Wks": tl((W["W_ks"] * WSC).astype(f8)),
        "Wvs": tl(W["W_vs"].astype(bf)),
        "Wqw": tl((W["W_qw"] * WSC).astype(f8)),
        "Wkw": tl((W["W_kw"] * WSC).astype(f8)),
        "Wvw": tl((W["W_vw"] * WSC).astype(f8)),
        "Wfc1": tl(W["W_fc1"].astype(bf)), "Wfc": tl(W["W_fc"].astype(bf), kc=8),
        "bqsT": bT(W["b_qs"], WSC), "bksT": bT(W["b_ks"]),
        "bqwT": bT(W["b_qw"], WSC),
        "bkwT": bT(W["b_kw"]), "bvsT": bT(W["b_vs"]),
        "bfc1T": bT(W["b_fc1"], 0.5), "bfcT": bT(W["b_fc"], 0.5),
        "bvw": W["b_vw"] * WSC,
    }
    in_maps = []
    for c in range(N_CORES):
        b, half = divmod(c, 2)
        blk = slice(half * NBH, half * NBH + NBH)
        ks_r = np.roll(k_s[b], -half * NBH, axis=0)
        vs_r = np.roll(v_s[b], -half * NBH, axis=0)
        m = dict(shared)
        m["qT"] = tl(q[b].T.astype(bf))
        kwT = k_w[b, blk].reshape(NTOK, D).T.astype(f8)
        vwT = v_w[b, blk].reshape(NTOK, D).T.astype(f8)

        def stg(a):
            a4 = a.reshape(4, 128, 4, 1024)            # [k, p, sg, t]
            return np.ascontiguousarray(a4.transpose(2, 1, 0, 3).reshape(4, 128, NTOK))

        m["kwT"] = stg(kwT)
        m["vwT"] = stg(vwT)
        m["ksT"] = tl(ks_r.T.astype(bf))
        m["vsT"] = tl(vs_r.T.astype(bf))
        in_maps.append(m)
    return in_maps


def run_cores(inputs, trace=False):
    nc = _get_nc()
    in_maps = make_in_maps(inputs)
    res = run_bass_kernel_spmd(nc, in_maps, list(range(N_CORES)), trace=trace)
    return res


def assemble(res):
    out = np.empty((B, LQ, D), dtype=np.float32)
    for b in range(B):
        out[b] = (
            res.results[2 * b]["outT"].astype(np.float32)
            + res.results[2 * b + 1]["outT"].astype(np.float32)
        ).T
    return out


def kernel(**inputs) -> np.ndarray:
    res = run_cores(inputs, trace=False)
    return assemble(res)


if __name__ == "__main__":
    import reference

    inp = {k: np.asarray(v) for k, v in reference.setup_inputs().items()}
    out = kernel(**inp)
    exp = np.asarray(reference.reference(**inp))
    err = np.abs(out - exp).max() / np.abs(exp).max()
    print("max rel err:", err)

